# revision 1
# baseline (speedup 1.0000x reference)
"""Multi-head causal attention (B=2, T=2048, C=1024, H=16, HS=64) on 8 TRN2
NeuronCores.

Sharding: 2 heads per core (tensor parallel). Each core receives the full
(pre-transposed) activations xT [B, C, T], its 2 heads' QKV weight slices
packed [C, 128], and its 128-column slice of w_proj transposed [128, C].
Each core computes a partial output [B, T, C]; the host sums the 8 partials
and adds b_proj.

Per-core kernel (all matmuls in float32r -- tf32-like, 1 cycle/row):
  - QT/KT/VT [128(2 heads x 64), T] via lhsT=weight chunks, rhs=xT chunks.
  - V_aug [keys, 128]: V (cols 0:64, via PE-transpose of VT) | ones (64:128).
  - Flash-style causal attention in transposed layout: S^T[keys, q] blocks
    via lhsT=KT block, rhs=QT slice; exp on ScalarE (no max subtraction --
    scores are O(1) by construction); O^T = [V|1].T @ P^T accumulated over
    key blocks gives both O rows (0:64) and the softmax sums l replicated
    (rows 64:128) in one pass.
  - Normalize with reciprocal_approx_fast + mixed-base tensor_tensor.
  - Output projection: lhsT = OhatT t-chunks, rhs = w_projT slice.

The two batches are software-pipelined: batch 1's QKV matmuls are emitted
between batch 0's attention groups so the PE always has independent work
while ScalarE (exp) catches up -- keeping the PE HAM clock at 2.4 GHz.
"""

import math
import sys
from contextlib import ExitStack

if "/opt/trn_rl_repo" not in sys.path:
    sys.path.insert(0, "/opt/trn_rl_repo")

import numpy as np

import concourse.mybir as mybir
import concourse.tile as tile
from concourse import bacc
from concourse.bass import ts
from concourse.bass_utils import run_bass_kernel_spmd
from concourse.tile_rust import add_dep_helper

B, T, C = 2, 2048, 1024
H, HS = 16, 64
NCORES = 8
HPC = H // NCORES  # heads per core
P = 128
G = 512  # q-group size
NG = T // G
KB = 128  # key block
NPO = C // P  # contraction chunks
F32 = mybir.dt.float32
F32R = mybir.dt.float32r
BF16 = mybir.dt.bfloat16

_nc_cache = {}


def _emit(tc):
    nc = tc.nc
    _last_pe = [None]
    xt = nc.dram_tensor("xt", [B, C, T], F32R, kind="ExternalInput").ap()
    wq2 = nc.dram_tensor("wq2", [C, 128], F32R, kind="ExternalInput").ap()
    wk2 = nc.dram_tensor("wk2", [C, 128], F32R, kind="ExternalInput").ap()
    wv2 = nc.dram_tensor("wv2", [C, 128], F32R, kind="ExternalInput").ap()
    wpt = nc.dram_tensor("wpt", [128, C], F32R, kind="ExternalInput").ap()
    tri = nc.dram_tensor("tri", [P, P], BF16, kind="ExternalInput").ap()
    identd = nc.dram_tensor("ident", [P, 64], BF16, kind="ExternalInput").ap()
    onesd = nc.dram_tensor("ones", [P, T // KB, 64], BF16, kind="ExternalInput").ap()
    out = nc.dram_tensor("out", [B, T, C], F32, kind="ExternalOutput").ap()

    ctx = ExitStack()
    persist = ctx.enter_context(tc.tile_pool(name="persist", bufs=1))
    xt_pool = ctx.enter_context(tc.tile_pool(name="xtp", bufs=4))
    qk_pool = ctx.enter_context(tc.tile_pool(name="qkp", bufs=2))
    vt_pool = ctx.enter_context(tc.tile_pool(name="vtp", bufs=2))
    vaug_pool = ctx.enter_context(tc.tile_pool(name="vaugp", bufs=2))
    pt_pool = ctx.enter_context(tc.tile_pool(name="ptp", bufs=4))
    norm_pool = ctx.enter_context(tc.tile_pool(name="normp", bufs=2))
    ohat_pool = ctx.enter_context(tc.tile_pool(name="ohatp", bufs=2))
    out_pool = ctx.enter_context(tc.tile_pool(name="outp", bufs=2))
    st_psum = ctx.enter_context(tc.tile_pool(name="stps", bufs=2, space="PSUM"))
    ot_psum = ctx.enter_context(tc.tile_pool(name="otps", bufs=2, space="PSUM"))
    mm_psum = ctx.enter_context(tc.tile_pool(name="mmps", bufs=2, space="PSUM"))

    wq_sb = persist.tile([P, NPO, 128], F32R, tag="wq")
    wk_sb = persist.tile([P, NPO, 128], F32R, tag="wk")
    wv_sb = persist.tile([P, NPO, 128], F32R, tag="wv")
    wpt_sb = persist.tile([P, C], F32R, tag="wpt")
    tri_sb = persist.tile([P, P], BF16, tag="tri")
    ident = persist.tile([P, 64], BF16, tag="ident")

    # ---- input loading: per-tg xT tiles, one 2MB DMA each ----
    def load_xt_tg(eng, b, tg, dep=None):
        t = xt_pool.tile([P, NPO, 512], F32R, tag="xt", name=f"xt{b}{tg}")
        i = eng.dma_start(
            t[:],
            xt[b, :, ts(tg, 512)].rearrange("(po pi) t -> pi po t", pi=P),
        )
        if dep is not None:
            add_dep_helper(i.ins, dep.ins, sync=True)
        return t, i

    nc.sync.dma_start(wq_sb[:], wq2.rearrange("(po pi) d -> pi po d", pi=P))
    nc.sync.dma_start(wk_sb[:], wk2.rearrange("(po pi) d -> pi po d", pi=P))
    nc.sync.dma_start(wv_sb[:], wv2.rearrange("(po pi) d -> pi po d", pi=P))
    nc.sync.dma_start(tri_sb[:], tri[:])
    nc.sync.dma_start(ident[:], identd[:])
    xt0, xt0_dmas = [], []
    for tg in range(NG):
        t, i = load_xt_tg(nc.sync, 0, tg, dep=xt0_dmas[tg - 2] if tg >= 2 else None)
        xt0.append(t)
        xt0_dmas.append(i)
        if tg == 1:
            nc.sync.dma_start(wpt_sb[:], wpt[:])

    def new_state(b, xts):
        st = {
            "b": b,
            "xt": xts,
            "qt": qk_pool.tile([P, T], F32R, tag="qt", name=f"qt{b}"),
            "kt": qk_pool.tile([P, T], F32R, tag="kt", name=f"kt{b}"),
            "vt": vt_pool.tile([P, T], BF16, tag="vt", name=f"vt{b}"),
            "ohat": ohat_pool.tile([P, T], F32R, tag="ohat", name=f"oh{b}"),
            "vaug": [],
        }
        for h in range(HPC):
            va = vaug_pool.tile(
                [P, T // KB, 128], BF16, tag=f"vaug{h}", name=f"va{b}{h}"
            )
            i = nc.gpsimd.dma_start(va[:, :, 64:128], onesd[:])
            add_dep_helper(i.ins, xt0_dmas[0].ins, sync=True)
            st["vaug"].append(va)
        return st

    def emit_qkv_group(st, which, tg):
        w_sb, dst = {
            "q": (wq_sb, st["qt"]),
            "k": (wk_sb, st["kt"]),
            "v": (wv_sb, st["vt"]),
        }[which]
        ps = mm_psum.tile([P, 512], F32, tag="mm", name=f"qkv{which}{tg}")
        for po in range(NPO):
            mi = nc.tensor.matmul(
                ps[:],
                w_sb[:, po, :],
                st["xt"][tg][:, po, :],
                start=(po == 0),
                stop=(po == NPO - 1),
            )
            _last_pe[0] = mi
        nc.vector.tensor_copy(dst[:, ts(tg, 512)], ps[:])

    def emit_vaug_part(st, kbg):
        vaug = st["vaug"]
        tps = [
            mm_psum.tile([P, 4, 64], BF16, tag="mm", name=f"vtr{h}")
            for h in range(HPC)
        ]
        for kk in range(4):
            kb = 4 * kbg + kk
            for h in range(HPC):
                nc.tensor.transpose(
                    tps[h][:, kk, :],
                    st["vt"][64 * h : 64 * h + 64, ts(kb, KB)],
                    ident[64 * h : 64 * h + 64, :],
                )
        for h in range(HPC):
            nc.vector.tensor_copy(
                vaug[h][:, 4 * kbg : 4 * kbg + 4, 0:64], tps[h][:]
            )

    def emit_attn_g(st, g):
        b, qt, kt, vaug, ohat = st["b"], st["qt"], st["kt"], st["vaug"], st["ohat"]
        l_sb = norm_pool.tile([P, G], F32, tag="lsb", name=f"l{b}{g}")
        rinv = norm_pool.tile([P, G], F32, tag="rinv", name=f"r{b}{g}")
        otps_h = [
            ot_psum.tile([P, G], F32, tag="ot", name=f"ot{h}") for h in range(HPC)
        ]
        n_j = 4 * g + 4
        for jg in range(math.ceil(n_j / 2)):
            js = [j for j in (2 * jg, 2 * jg + 1) if j < n_j]
            diag = 2 * jg >= 4 * g
            stps_h = [
                st_psum.tile([P, 2, G], F32, tag="st", name=f"st{h}")
                for h in range(HPC)
            ]
            pt_h = [
                pt_pool.tile([P, 2, G], BF16, tag="pt", name=f"pt{h}")
                for h in range(HPC)
            ]
            for idx, j in enumerate(js):
                r = j - 4 * g
                q0 = 128 * r if r >= 0 else 0
                for h in range(HPC):
                    hb = 64 * h
                    nc.tensor.matmul(
                        stps_h[h][:, idx, q0:G],
                        kt[hb : hb + 64, ts(j, KB)],
                        qt[hb : hb + 64, G * g + q0 : G * (g + 1)],
                        start=True,
                        stop=True,
                    )
            for h in range(HPC):
                stps, pt = stps_h[h], pt_h[h]
                if not diag:
                    nc.scalar.activation(
                        pt[:, :, :],
                        stps[:, :, :],
                        mybir.ActivationFunctionType.Exp,
                        scale=float(HS) ** -0.5,
                    )
                else:
                    for idx, j in enumerate(js):
                        q0 = 128 * (j - 4 * g)
                        nc.scalar.activation(
                            pt[:, idx, q0:G],
                            stps[:, idx, q0:G],
                            mybir.ActivationFunctionType.Exp,
                            scale=float(HS) ** -0.5,
                        )
                        nc.vector.tensor_tensor(
                            pt[:, idx, q0 : q0 + 128],
                            pt[:, idx, q0 : q0 + 128],
                            tri_sb[:],
                            mybir.AluOpType.mult,
                        )
            for idx, j in enumerate(js):
                r = j - 4 * g
                q0 = 128 * r if r >= 0 else 0
                for h in range(HPC):
                    nc.tensor.matmul(
                        otps_h[h][:, q0:G],
                        vaug[h][:, j, :],
                        pt_h[h][:, idx, q0:G],
                        start=(j == 0),
                        stop=(j == n_j - 1),
                    )
        stag = norm_pool.tile([P, G], F32, tag="stag", name=f"sg{b}{g}")
        for h in range(HPC):
            hb = 64 * h
            nc.vector.tensor_copy(stag[hb : hb + 64, :], otps_h[h][0:64, :])
            nc.vector.tensor_copy(l_sb[hb : hb + 64, :], otps_h[h][64:128, :])
        nc.vector.reciprocal_approx_fast(rinv[:], l_sb[:])
        nc.vector.tensor_tensor(
            ohat[:, ts(g, G)], stag[:], rinv[:], mybir.AluOpType.mult
        )

    def emit_proj(st, g):
        b, ohat = st["b"], st["ohat"]
        for tc4 in range(G // P):
            t0 = G * g + P * tc4
            o_sb = out_pool.tile([P, C], F32, tag="osb", name=f"osb{tc4}")
            for n in range(C // 512):
                pj = mm_psum.tile([P, 512], F32, tag="mm", name=f"pj{n}")
                nc.tensor.matmul(
                    pj[:],
                    ohat[:, t0 : t0 + P],
                    wpt_sb[:, ts(n, 512)],
                    start=True,
                    stop=True,
                )
                if (2 * tc4 + n) % 2 == 0:
                    nc.vector.tensor_copy(o_sb[:, ts(n, 512)], pj[:])
                else:
                    nc.scalar.copy(o_sb[:, ts(n, 512)], pj[:])
            nc.sync.dma_start(out[b, t0 : t0 + P, :], o_sb[:])

    # ================= pipelined emission =================
    st0 = new_state(0, xt0)
    st1 = new_state(1, [None] * NG)
    xt1_dmas = []
    for tg in range(NG):
        for which in ("q", "k", "v"):
            emit_qkv_group(st0, which, tg)
        emit_vaug_part(st0, tg)
        emit_attn_g(st0, tg)
        t, i = load_xt_tg(
            nc.gpsimd, 1, tg,
            dep=xt1_dmas[tg - 2] if tg >= 2 else xt0_dmas[NG - 1],
        )
        st1["xt"][tg] = t
        xt1_dmas.append(i)
        if tg >= 1:
            for which in ("q", "k", "v"):
                emit_qkv_group(st1, which, tg - 1)
    for which in ("q", "k", "v"):
        emit_qkv_group(st1, which, NG - 1)
    for g in range(NG):
        emit_vaug_part(st1, g)
        emit_attn_g(st1, g)
        emit_proj(st0, g)
        if g >= 1:
            emit_proj(st1, g - 1)
    emit_proj(st1, NG - 1)
    ctx.close()


def _build():
    if "nc" in _nc_cache:
        return _nc_cache["nc"]
    nc = bacc.Bacc("TRN2", target_bir_lowering=False, debug=False)
    with tile.TileContext(nc) as tc:
        _emit(tc)
    nc.compile()
    _nc_cache["nc"] = nc
    return nc


def _make_in_maps(x, wq, wk, wv, w_proj):
    xt = np.ascontiguousarray(x.transpose(0, 2, 1)).astype(np.float32)
    import ml_dtypes

    tri = np.triu(np.ones((P, P), dtype=np.float32)).astype(ml_dtypes.bfloat16)
    ident = np.tile(np.eye(64, dtype=np.float32), (2, 1)).astype(ml_dtypes.bfloat16)
    ones = np.ones((P, T // KB, 64), dtype=np.float32).astype(ml_dtypes.bfloat16)
    in_maps = []
    for c in range(NCORES):
        h0 = HPC * c
        in_maps.append(
            {
                "xt": xt,
                "wq2": np.ascontiguousarray(
                    np.concatenate([wq[h0 + i] for i in range(HPC)], axis=1)
                ).astype(np.float32),
                "wk2": np.ascontiguousarray(
                    np.concatenate([wk[h0 + i] for i in range(HPC)], axis=1)
                ).astype(np.float32),
                "wv2": np.ascontiguousarray(
                    np.concatenate([wv[h0 + i] for i in range(HPC)], axis=1)
                ).astype(np.float32),
                "wpt": np.ascontiguousarray(
                    w_proj[:, 128 * c : 128 * (c + 1)].T
                ).astype(np.float32),
                "tri": tri,
                "ident": ident,
                "ones": ones,
            }
        )
    return in_maps


def kernel(x, wq, wk, wv, w_proj, b_proj):
    x = np.asarray(x, dtype=np.float32)
    wq = np.asarray(wq, dtype=np.float32)
    wk = np.asarray(wk, dtype=np.float32)
    wv = np.asarray(wv, dtype=np.float32)
    w_proj = np.asarray(w_proj, dtype=np.float32)
    b_proj = np.asarray(b_proj, dtype=np.float32)

    nc = _build()
    in_maps = _make_in_maps(x, wq, wk, wv, w_proj)
    res = run_bass_kernel_spmd(nc, in_maps, core_ids=list(range(NCORES)))
    acc = np.zeros((B, T, C), dtype=np.float64)
    for r in res.results:
        acc += r["out"]
    return (acc + b_proj).astype(np.float32)



# revision 2
# speedup vs baseline: 1.3925x; 1.3925x over previous
"""Multi-head causal attention (B=2, T=2048, C=1024, H=16, HS=64) on 8 TRN2
NeuronCores.

Sharding: hybrid batch x head-group tensor parallel. Core c handles batch
c//4 and heads 4*(c%4) .. 4*(c%4)+3, processed as two head-PAIRS (A, B)
that are software-pipelined against each other. Each core loads only its
batch's activations once (bf16, SBUF-resident), computes a partial output
out_t[C, T] via its 256 rows of w_proj, and the host sums 4 partials per
batch (+ bias) and transposes.

Per-core kernel (bf16 matmuls, fp32 PSUM accumulate):
  - ~12 warm-up matmuls on the first-arriving weight tile keep the PE HAM
    clock at 2.4 GHz through the initial DMA fill; a dummy exp preloads
    the ScalarE activation table.
  - QT/KT/VT [128(2 heads x 64), T] per pair: lhsT=weight chunks (bf16),
    rhs=resident xT chunks.
  - V_aug [keys, 128]: V (cols 0:64 via PE-transpose of VT) | ones.
  - Flash-style causal attention in transposed layout: S^T[keys, q] via
    lhsT=KT block (K=64; the two heads' matmuls occupy disjoint PE row
    groups and run concurrently), exp on ScalarE, O^T accumulated via
    lhsT=V_aug — rows 64:128 give the softmax sums. OT for block-pair jg
    is emitted during jg+1 so the PE never head-of-line blocks on exp.
  - Pair B's QKV matmuls are interleaved between pair A's attention
    steps; the (transposed) output projection lhsT=w_projT chunks,
    rhs=OhatT is interleaved between pair B's attention steps.
"""

import math
import sys
from contextlib import ExitStack

if "/opt/trn_rl_repo" not in sys.path:
    sys.path.insert(0, "/opt/trn_rl_repo")

import numpy as np

import concourse.mybir as mybir
import concourse.tile as tile
from concourse import bacc
from concourse.bass import ts
from concourse.bass_utils import run_bass_kernel_spmd

B, T, C = 2, 2048, 1024
H, HS = 16, 64
NCORES = 8
P = 128
G = 512  # q-group size
NG = T // G
KB = 128  # key block
NPO = C // P  # contraction chunks
F32 = mybir.dt.float32
BF16 = mybir.dt.bfloat16
EXP = mybir.ActivationFunctionType.Exp
MULT = mybir.AluOpType.mult

_nc_cache = {}


def _emit(tc):
    nc = tc.nc
    xt = nc.dram_tensor("xt", [C, T], BF16, kind="ExternalInput").ap()
    wq2 = nc.dram_tensor("wq2", [C, 2 * P], BF16, kind="ExternalInput").ap()
    wk2 = nc.dram_tensor("wk2", [C, 2 * P], BF16, kind="ExternalInput").ap()
    wv2 = nc.dram_tensor("wv2", [C, 2 * P], BF16, kind="ExternalInput").ap()
    wptd = nc.dram_tensor("wpt", [2 * P, C], BF16, kind="ExternalInput").ap()
    trid = nc.dram_tensor("tri", [P, P], BF16, kind="ExternalInput").ap()
    identd = nc.dram_tensor("ident", [P, 64], BF16, kind="ExternalInput").ap()
    onesd = nc.dram_tensor("ones", [P, T // KB, 64], BF16, kind="ExternalInput").ap()
    out = nc.dram_tensor("out", [C, T], F32, kind="ExternalOutput").ap()

    ctx = ExitStack()
    persist = ctx.enter_context(tc.tile_pool(name="persist", bufs=1))
    qk_pool = ctx.enter_context(tc.tile_pool(name="qkp", bufs=2))
    vt_pool = ctx.enter_context(tc.tile_pool(name="vtp", bufs=2))
    vaug_pool = ctx.enter_context(tc.tile_pool(name="vaugp", bufs=2))
    pt_pool = ctx.enter_context(tc.tile_pool(name="ptp", bufs=3))
    norm_pool = ctx.enter_context(tc.tile_pool(name="normp", bufs=2))
    ohat_pool = ctx.enter_context(tc.tile_pool(name="ohatp", bufs=2))
    out_pool = ctx.enter_context(tc.tile_pool(name="outp", bufs=4))
    st_psum = ctx.enter_context(tc.tile_pool(name="stps", bufs=2, space="PSUM"))
    ot_psum = ctx.enter_context(tc.tile_pool(name="otps", bufs=2, space="PSUM"))
    mm_psum = ctx.enter_context(tc.tile_pool(name="mmps", bufs=2, space="PSUM"))

    xt_sb = persist.tile([P, NPO, T], BF16, tag="xt")
    wq_sb = persist.tile([P, NPO, 2 * P], BF16, tag="wq")
    wk_sb = persist.tile([P, NPO, 2 * P], BF16, tag="wk")
    wv_sb = persist.tile([P, NPO, 2 * P], BF16, tag="wv")
    wpt_sb = persist.tile([P, 2, C], BF16, tag="wpt")
    tri_sb = persist.tile([P, P], BF16, tag="tri")
    ident = persist.tile([P, 64], BF16, tag="ident")

    # ---- DMA schedule: wq first (enables warm-up), then xt chunks ----
    nc.sync.dma_start(wq_sb[:], wq2.rearrange("(po pi) d -> pi po d", pi=P))

    # PE warm-up during the DMA fill: keeps the HAM clock releasing to
    # 2.4 GHz before the first real matmul. Also preload the exp table.
    warm_ps = mm_psum.tile([P, G], F32, tag="mm", name="warm")
    for _ in range(12):
        nc.tensor.matmul(
            warm_ps[:], wq_sb[:, 0, 0:P], wq_sb[:, 0:2, :], start=True, stop=True
        )
    dummy_act = norm_pool.tile([P, 8], F32, tag="dact", name="dummy_act")
    nc.scalar.activation(dummy_act[:], wq_sb[:, 0, 0:8], EXP, scale=0.01)

    nc.sync.dma_start(
        xt_sb[:, :, ts(0, G)],
        xt[:, ts(0, G)].rearrange("(po pi) t -> pi po t", pi=P),
    )
    nc.sync.dma_start(wk_sb[:], wk2.rearrange("(po pi) d -> pi po d", pi=P))
    nc.sync.dma_start(wv_sb[:], wv2.rearrange("(po pi) d -> pi po d", pi=P))
    for tg in range(1, NG):
        nc.sync.dma_start(
            xt_sb[:, :, ts(tg, G)],
            xt[:, ts(tg, G)].rearrange("(po pi) t -> pi po t", pi=P),
        )
    nc.gpsimd.dma_start(ident[:], identd[:])
    nc.gpsimd.dma_start(tri_sb[:], trid[:])

    def new_pair(p):
        st = {
            "p": p,
            "qt": qk_pool.tile([P, T], BF16, tag="qt", name=f"qt{p}"),
            "kt": qk_pool.tile([P, T], BF16, tag="kt", name=f"kt{p}"),
            "vt": vt_pool.tile([P, T], BF16, tag="vt", name=f"vt{p}"),
            "ohat": ohat_pool.tile([P, T], BF16, tag="ohat", name=f"oh{p}"),
            "vaug": [],
        }
        for h in range(2):
            va = vaug_pool.tile(
                [P, T // KB, 128], BF16, tag=f"vaug{h}", name=f"va{p}{h}"
            )
            nc.gpsimd.dma_start(va[:, :, 64:128], onesd[:])
            st["vaug"].append(va)
        return st

    stA = new_pair(0)
    stB = new_pair(1)
    nc.gpsimd.dma_start(wpt_sb[:], wptd.rearrange("(p pi) c -> pi p c", pi=P))

    def emit_qkv(st, which, tg):
        w_sb, dst = {
            "q": (wq_sb, st["qt"]),
            "k": (wk_sb, st["kt"]),
            "v": (wv_sb, st["vt"]),
        }[which]
        sl = slice(P * st["p"], P * st["p"] + P)
        ps = mm_psum.tile([P, G], F32, tag="mm", name=f"qkv{which}{tg}")
        for po in range(NPO):
            nc.tensor.matmul(
                ps[:],
                w_sb[:, po, sl],
                xt_sb[:, po, ts(tg, G)],
                start=(po == 0),
                stop=(po == NPO - 1),
            )
        nc.vector.tensor_copy(dst[:, ts(tg, G)], ps[:])

    def emit_vaug(st, tg):
        tps = [
            mm_psum.tile([P, 4, 64], BF16, tag="mm", name=f"vtr{h}")
            for h in range(2)
        ]
        for kk in range(4):
            kb = 4 * tg + kk
            for h in range(2):
                nc.tensor.transpose(
                    tps[h][:, kk, :],
                    st["vt"][64 * h : 64 * h + 64, ts(kb, KB)],
                    ident[64 * h : 64 * h + 64, :],
                )
        for h in range(2):
            nc.vector.tensor_copy(st["vaug"][h][:, 4 * tg : 4 * tg + 4, 0:64], tps[h][:])

    def emit_ot(st, g, otps, js, pt_h):
        n_j = 4 * g + 4
        for idx, j in enumerate(js):
            r = j - 4 * g
            q0 = P * r if r >= 0 else 0
            for h in range(2):
                nc.tensor.matmul(
                    otps[h][:, q0:G],
                    st["vaug"][h][:, j, :],
                    pt_h[h][:, idx, q0:G],
                    start=(j == 0),
                    stop=(j == n_j - 1),
                )

    def gen_attn(st, g):
        p, qt, kt, ohat = st["p"], st["qt"], st["kt"], st["ohat"]
        n_j = 4 * g + 4
        njg = (n_j + 1) // 2
        otps = [
            ot_psum.tile([P, G], F32, tag="ot", name=f"ot{p}{g}{h}") for h in range(2)
        ]
        prev = None
        for jg in range(njg):
            js = [j for j in (2 * jg, 2 * jg + 1) if j < n_j]
            diag = 2 * jg >= 4 * g
            stps_h = [
                st_psum.tile([P, 2, G], F32, tag="st", name=f"st{p}{g}{h}")
                for h in range(2)
            ]
            pt_h = [
                pt_pool.tile([P, 2, G], BF16, tag=f"pt{h}", name=f"pt{p}{g}{h}")
                for h in range(2)
            ]
            for idx, j in enumerate(js):
                r = j - 4 * g
                q0 = P * r if r >= 0 else 0
                for h in range(2):
                    hb = 64 * h
                    nc.tensor.matmul(
                        stps_h[h][:, idx, q0:G],
                        kt[hb : hb + 64, ts(j, KB)],
                        qt[hb : hb + 64, G * g + q0 : G * (g + 1)],
                        start=True,
                        stop=True,
                    )
            for h in range(2):
                if not diag:
                    nc.scalar.activation(
                        pt_h[h][:, :, :],
                        stps_h[h][:, :, :],
                        EXP,
                        scale=float(HS) ** -0.5,
                    )
                else:
                    for idx, j in enumerate(js):
                        q0 = P * (j - 4 * g)
                        nc.scalar.activation(
                            pt_h[h][:, idx, q0:G],
                            stps_h[h][:, idx, q0:G],
                            EXP,
                            scale=float(HS) ** -0.5,
                        )
                        nc.vector.tensor_tensor(
                            pt_h[h][:, idx, q0 : q0 + P],
                            pt_h[h][:, idx, q0 : q0 + P],
                            tri_sb[:],
                            MULT,
                        )
            if prev is not None:
                emit_ot(st, g, otps, *prev)
            prev = (js, pt_h)
            yield
        emit_ot(st, g, otps, *prev)
        l_sb = norm_pool.tile([P, G], F32, tag="lsb", name=f"l{p}{g}")
        rinv = norm_pool.tile([P, G], F32, tag="rinv", name=f"r{p}{g}")
        stag = norm_pool.tile([P, G], F32, tag="stag", name=f"sg{p}{g}")
        for h in range(2):
            hb = 64 * h
            nc.vector.tensor_copy(stag[hb : hb + 64, :], otps[h][0:64, :])
            nc.vector.tensor_copy(l_sb[hb : hb + 64, :], otps[h][64:128, :])
        nc.vector.reciprocal_approx_fast(rinv[:], l_sb[:])
        nc.vector.tensor_tensor(ohat[:, ts(g, G)], stag[:], rinv[:], MULT)
        yield

    def gen_qkv_pair(st):
        for tg in range(NG):
            for which in ("q", "k", "v"):
                emit_qkv(st, which, tg)
                yield
            emit_vaug(st, tg)
            yield

    def gen_proj(tg):
        for cc in range(NPO):
            ps = mm_psum.tile([P, G], F32, tag="mm", name=f"pj{tg}{cc}")
            for pp in range(2):
                nc.tensor.matmul(
                    ps[:],
                    wpt_sb[:, pp, ts(cc, P)],
                    [stA, stB][pp]["ohat"][:, ts(tg, G)],
                    start=(pp == 0),
                    stop=(pp == 1),
                )
            o_sb = out_pool.tile([P, G], F32, tag="osb", name=f"osb{tg}{cc}")
            if cc % 2 == 0:
                nc.vector.tensor_copy(o_sb[:], ps[:])
            else:
                nc.scalar.copy(o_sb[:], ps[:])
            nc.sync.dma_start(out[ts(cc, P), ts(tg, G)], o_sb[:])
            if cc % 2 == 1:
                yield

    def advance(gens):
        while gens:
            try:
                next(gens[0])
                return
            except StopIteration:
                gens.pop(0)

    # ---- phase 1: QKV + V_aug for pair A (gated by xt DMA arrival) ----
    for _ in gen_qkv_pair(stA):
        pass

    # ---- phase 2: attention A, interleaved with QKV B ----
    qb = [gen_qkv_pair(stB)]
    for g in range(NG):
        for _ in gen_attn(stA, g):
            advance(qb)
    while qb:
        advance(qb)

    # ---- phase 3: attention B, interleaved with proj (needs both ohats) ----
    pj = []
    for g in range(NG):
        for _ in gen_attn(stB, g):
            advance(pj)
        pj.append(gen_proj(g))
    while pj:
        advance(pj)

    ctx.close()


def _build():
    if "nc" in _nc_cache:
        return _nc_cache["nc"]
    nc = bacc.Bacc("TRN2", target_bir_lowering=False, debug=False)
    with tile.TileContext(nc) as tc:
        _emit(tc)
    nc.compile()
    _nc_cache["nc"] = nc
    return nc


def _make_in_maps(x, wq, wk, wv, w_proj):
    import ml_dtypes

    bf = ml_dtypes.bfloat16
    tri = np.triu(np.ones((P, P), dtype=np.float32)).astype(bf)
    ident = np.tile(np.eye(64, dtype=np.float32), (2, 1)).astype(bf)
    ones = np.ones((P, T // KB, 64), dtype=np.float32).astype(bf)
    xts = [
        np.ascontiguousarray(np.asarray(x[b], np.float32).T).astype(bf)
        for b in range(B)
    ]
    in_maps = []
    for c in range(NCORES):
        b, hp = divmod(c, 4)
        h0 = 4 * hp
        in_maps.append(
            {
                "xt": xts[b],
                "wq2": np.ascontiguousarray(
                    np.concatenate([wq[h0 + i] for i in range(4)], axis=1)
                ).astype(bf),
                "wk2": np.ascontiguousarray(
                    np.concatenate([wk[h0 + i] for i in range(4)], axis=1)
                ).astype(bf),
                "wv2": np.ascontiguousarray(
                    np.concatenate([wv[h0 + i] for i in range(4)], axis=1)
                ).astype(bf),
                "wpt": np.ascontiguousarray(
                    w_proj[:, 256 * hp : 256 * (hp + 1)].T
                ).astype(bf),
                "tri": tri,
                "ident": ident,
                "ones": ones,
            }
        )
    return in_maps


def kernel(x, wq, wk, wv, w_proj, b_proj):
    x = np.asarray(x, dtype=np.float32)
    wq = np.asarray(wq, dtype=np.float32)
    wk = np.asarray(wk, dtype=np.float32)
    wv = np.asarray(wv, dtype=np.float32)
    w_proj = np.asarray(w_proj, dtype=np.float32)
    b_proj = np.asarray(b_proj, dtype=np.float32)

    nc = _build()
    in_maps = _make_in_maps(x, wq, wk, wv, w_proj)
    res = run_bass_kernel_spmd(nc, in_maps, core_ids=list(range(NCORES)))
    acc = np.zeros((B, C, T), dtype=np.float64)
    for c, r in enumerate(res.results):
        acc[c // 4] += r["out"]
    return (acc.transpose(0, 2, 1) + b_proj).astype(np.float32)


# revision 8
# speedup vs baseline: 1.4445x; 1.0374x over previous
"""Multi-head causal attention (B=2, T=2048, C=1024, H=16, HS=64) on 8 TRN2
NeuronCores.

Sharding: hybrid batch x head-group tensor parallel. Core c handles batch
c//4 and heads 4*(c%4) .. 4*(c%4)+3, processed as two head-PAIRS (A, B)
that are software-pipelined against each other. Each core loads only its
batch's activations once (bf16, SBUF-resident), computes a partial output
out_t[C, T] via its 256 rows of w_proj, and the host sums 4 partials per
batch (+ bias) and transposes.

Per-core kernel (bf16 matmuls, fp32 PSUM accumulate):
  - ~12 warm-up matmuls on the first-arriving weight tile keep the PE HAM
    clock at 2.4 GHz through the initial DMA fill; a dummy exp preloads
    the ScalarE activation table.
  - QT/KT/VT [128(2 heads x 64), T] per pair: lhsT=weight chunks (bf16),
    rhs=resident xT chunks.
  - V_aug [keys, 128]: V (cols 0:64 via PE-transpose of VT) | ones.
  - Flash-style causal attention in transposed layout: S^T[keys, q] via
    lhsT=KT block (K=64; the two heads' matmuls occupy disjoint PE row
    groups and run concurrently), exp on ScalarE, O^T accumulated via
    lhsT=V_aug — rows 64:128 give the softmax sums. OT for block-pair jg
    is emitted during jg+1 so the PE never head-of-line blocks on exp.
  - Pair B's QKV matmuls are interleaved between pair A's attention
    steps; the (transposed) output projection lhsT=w_projT chunks,
    rhs=OhatT is interleaved between pair B's attention steps.
"""

import math
import sys
from contextlib import ExitStack

if "/opt/trn_rl_repo" not in sys.path:
    sys.path.insert(0, "/opt/trn_rl_repo")

import numpy as np

import concourse.mybir as mybir
import concourse.tile as tile
from concourse import bacc
from concourse.bass import ts
from concourse.bass_utils import run_bass_kernel_spmd

B, T, C = 2, 2048, 1024
H, HS = 16, 64
NCORES = 8
P = 128
G = 512  # q-group size
NG = T // G
KB = 128  # key block
NPO = C // P  # contraction chunks
F32 = mybir.dt.float32
BF16 = mybir.dt.bfloat16
EXP = mybir.ActivationFunctionType.Exp
MULT = mybir.AluOpType.mult

_nc_cache = {}


def _emit(tc):
    nc = tc.nc
    # All inputs are host-pre-arranged so every DMA descriptor moves a
    # 4-32 KB contiguous row (descriptor-count, not bandwidth, limits the
    # small strided layout).
    xt = nc.dram_tensor("xt", [P, NG, NPO, G], BF16, kind="ExternalInput").ap()
    wq2 = nc.dram_tensor("wq2", [P, NPO, 2 * P], BF16, kind="ExternalInput").ap()
    wk2 = nc.dram_tensor("wk2", [P, NPO, 2 * P], BF16, kind="ExternalInput").ap()
    wv2 = nc.dram_tensor("wv2", [P, NPO, 2 * P], BF16, kind="ExternalInput").ap()
    wptd = nc.dram_tensor("wpt", [P, 2, C], BF16, kind="ExternalInput").ap()
    trid = nc.dram_tensor("tri", [P, P], BF16, kind="ExternalInput").ap()
    identd = nc.dram_tensor("ident", [P, 64], BF16, kind="ExternalInput").ap()
    onesd = nc.dram_tensor("ones", [P, T // KB, 64], BF16, kind="ExternalInput").ap()
    out = nc.dram_tensor("out", [C, T], F32, kind="ExternalOutput").ap()

    ctx = ExitStack()
    persist = ctx.enter_context(tc.tile_pool(name="persist", bufs=1))
    qk_pool = ctx.enter_context(tc.tile_pool(name="qkp", bufs=2))
    vt_pool = ctx.enter_context(tc.tile_pool(name="vtp", bufs=2))
    vaug_pool = ctx.enter_context(tc.tile_pool(name="vaugp", bufs=2))
    pt_pool = ctx.enter_context(tc.tile_pool(name="ptp", bufs=3))
    norm_pool = ctx.enter_context(tc.tile_pool(name="normp", bufs=2))
    ohat_pool = ctx.enter_context(tc.tile_pool(name="ohatp", bufs=2))
    out_pool = ctx.enter_context(tc.tile_pool(name="outp", bufs=4))
    st_psum = ctx.enter_context(tc.tile_pool(name="stps", bufs=2, space="PSUM"))
    ot_psum = ctx.enter_context(tc.tile_pool(name="otps", bufs=2, space="PSUM"))
    mm_psum = ctx.enter_context(tc.tile_pool(name="mmps", bufs=2, space="PSUM"))

    xt_sb = persist.tile([P, NG, NPO, G], BF16, tag="xt")
    wq_sb = persist.tile([P, NPO, 2 * P], BF16, tag="wq")
    wk_sb = persist.tile([P, NPO, 2 * P], BF16, tag="wk")
    wv_sb = persist.tile([P, NPO, 2 * P], BF16, tag="wv")
    wpt_sb = persist.tile([P, 2, C], BF16, tag="wpt")
    tri_sb = persist.tile([P, P], BF16, tag="tri")
    ident = persist.tile([P, 64], BF16, tag="ident")

    # ---- DMA schedule: wq first (enables warm-up), then xt chunks ----
    nc.sync.dma_start(wq_sb[:], wq2[:])

    # PE warm-up during the DMA fill: keeps the HAM clock releasing to
    # 2.4 GHz before the first real matmul. Also preload the exp table.
    warm_ps = mm_psum.tile([P, G], F32, tag="mm", name="warm")
    for _ in range(12):
        nc.tensor.matmul(
            warm_ps[:], wq_sb[:, 0, 0:P], wq_sb[:, 0:2, :], start=True, stop=True
        )
    dummy_act = norm_pool.tile([P, 8], F32, tag="dact", name="dummy_act")
    nc.scalar.activation(dummy_act[:], wq_sb[:, 0, 0:8], EXP, scale=0.01)

    nc.sync.dma_start(xt_sb[:, 0, :, :], xt[:, 0, :, :])
    nc.sync.dma_start(wk_sb[:], wk2[:])
    nc.sync.dma_start(wv_sb[:], wv2[:])
    for tg in range(1, NG):
        nc.sync.dma_start(xt_sb[:, tg, :, :], xt[:, tg, :, :])
    nc.gpsimd.dma_start(ident[:], identd[:])
    nc.gpsimd.dma_start(tri_sb[:], trid[:])

    def new_pair(p):
        st = {
            "p": p,
            "qt": qk_pool.tile([P, T], BF16, tag="qt", name=f"qt{p}"),
            "kt": qk_pool.tile([P, T], BF16, tag="kt", name=f"kt{p}"),
            "vt": vt_pool.tile([P, T], BF16, tag="vt", name=f"vt{p}"),
            "ohat": ohat_pool.tile([P, T], BF16, tag="ohat", name=f"oh{p}"),
            "vaug": [],
        }
        for h in range(2):
            va = vaug_pool.tile(
                [P, T // KB, 128], BF16, tag=f"vaug{h}", name=f"va{p}{h}"
            )
            nc.gpsimd.dma_start(va[:, :, 64:128], onesd[:])
            st["vaug"].append(va)
        return st

    stA = new_pair(0)
    stB = new_pair(1)
    nc.gpsimd.dma_start(wpt_sb[:], wptd[:])

    def emit_qkv(st, which, tg):
        w_sb, dst = {
            "q": (wq_sb, st["qt"]),
            "k": (wk_sb, st["kt"]),
            "v": (wv_sb, st["vt"]),
        }[which]
        sl = slice(P * st["p"], P * st["p"] + P)
        ps = mm_psum.tile([P, G], F32, tag="mm", name=f"qkv{which}{tg}")
        for po in range(NPO):
            nc.tensor.matmul(
                ps[:],
                w_sb[:, po, sl],
                xt_sb[:, tg, po, :],
                start=(po == 0),
                stop=(po == NPO - 1),
            )
        nc.vector.tensor_copy(dst[:, ts(tg, G)], ps[:])

    def emit_vaug(st, tg):
        tps = [
            mm_psum.tile([P, 4, 64], BF16, tag="mm", name=f"vtr{h}")
            for h in range(2)
        ]
        for kk in range(4):
            kb = 4 * tg + kk
            for h in range(2):
                nc.tensor.transpose(
                    tps[h][:, kk, :],
                    st["vt"][64 * h : 64 * h + 64, ts(kb, KB)],
                    ident[64 * h : 64 * h + 64, :],
                )
        for h in range(2):
            nc.vector.tensor_copy(st["vaug"][h][:, 4 * tg : 4 * tg + 4, 0:64], tps[h][:])

    def emit_ot(st, g, otps, js, pt_h):
        n_j = 4 * g + 4
        for idx, j in enumerate(js):
            r = j - 4 * g
            q0 = P * r if r >= 0 else 0
            for h in range(2):
                nc.tensor.matmul(
                    otps[h][:, q0:G],
                    st["vaug"][h][:, j, :],
                    pt_h[h][:, idx, q0:G],
                    start=(j == 0),
                    stop=(j == n_j - 1),
                )

    def gen_attn(st, g):
        p, qt, kt, ohat = st["p"], st["qt"], st["kt"], st["ohat"]
        n_j = 4 * g + 4
        njg = (n_j + 1) // 2
        otps = [
            ot_psum.tile([P, G], F32, tag="ot", name=f"ot{p}{g}{h}") for h in range(2)
        ]
        prev = None
        for jg in range(njg):
            js = [j for j in (2 * jg, 2 * jg + 1) if j < n_j]
            diag = 2 * jg >= 4 * g
            stps_h = [
                st_psum.tile([P, 2, G], F32, tag="st", name=f"st{p}{g}{h}")
                for h in range(2)
            ]
            pt_h = [
                pt_pool.tile([P, 2, G], BF16, tag=f"pt{h}", name=f"pt{p}{g}{h}")
                for h in range(2)
            ]
            for idx, j in enumerate(js):
                r = j - 4 * g
                q0 = P * r if r >= 0 else 0
                for h in range(2):
                    hb = 64 * h
                    nc.tensor.matmul(
                        stps_h[h][:, idx, q0:G],
                        kt[hb : hb + 64, ts(j, KB)],
                        qt[hb : hb + 64, G * g + q0 : G * (g + 1)],
                        start=True,
                        stop=True,
                    )
            for h in range(2):
                if not diag:
                    nc.scalar.activation(
                        pt_h[h][:, :, :],
                        stps_h[h][:, :, :],
                        EXP,
                        scale=float(HS) ** -0.5,
                    )
                else:
                    for idx, j in enumerate(js):
                        q0 = P * (j - 4 * g)
                        nc.scalar.activation(
                            pt_h[h][:, idx, q0:G],
                            stps_h[h][:, idx, q0:G],
                            EXP,
                            scale=float(HS) ** -0.5,
                        )
                        nc.vector.tensor_tensor(
                            pt_h[h][:, idx, q0 : q0 + P],
                            pt_h[h][:, idx, q0 : q0 + P],
                            tri_sb[:],
                            MULT,
                        )
            if prev is not None:
                emit_ot(st, g, otps, *prev)
            prev = (js, pt_h)
            yield
        emit_ot(st, g, otps, *prev)
        l_sb = norm_pool.tile([P, G], F32, tag="lsb", name=f"l{p}{g}")
        rinv = norm_pool.tile([P, G], F32, tag="rinv", name=f"r{p}{g}")
        stag = norm_pool.tile([P, G], F32, tag="stag", name=f"sg{p}{g}")
        for h in range(2):
            hb = 64 * h
            nc.vector.tensor_copy(stag[hb : hb + 64, :], otps[h][0:64, :])
            nc.vector.tensor_copy(l_sb[hb : hb + 64, :], otps[h][64:128, :])
        nc.vector.reciprocal_approx_fast(rinv[:], l_sb[:])
        nc.vector.tensor_tensor(ohat[:, ts(g, G)], stag[:], rinv[:], MULT)
        yield

    def gen_qkv_tg(st, tg):
        for which in ("q", "k", "v"):
            emit_qkv(st, which, tg)
            yield
        emit_vaug(st, tg)
        yield

    def gen_proj(tg):
        for cc in range(NPO):
            ps = mm_psum.tile([P, G], F32, tag="mm", name=f"pj{tg}{cc}")
            for pp in range(2):
                nc.tensor.matmul(
                    ps[:],
                    wpt_sb[:, pp, ts(cc, P)],
                    [stA, stB][pp]["ohat"][:, ts(tg, G)],
                    start=(pp == 0),
                    stop=(pp == 1),
                )
            o_sb = out_pool.tile([P, G], F32, tag="osb", name=f"osb{tg}{cc}")
            if cc % 2 == 0:
                nc.vector.tensor_copy(o_sb[:], ps[:])
                nc.sync.dma_start(out[ts(cc, P), ts(tg, G)], o_sb[:])
            else:
                nc.scalar.copy(o_sb[:], ps[:])
                nc.gpsimd.dma_start(out[ts(cc, P), ts(tg, G)], o_sb[:])
            if cc % 2 == 1:
                yield

    def advance(gens):
        while gens:
            try:
                next(gens[0])
                return
            except StopIteration:
                gens.pop(0)

    def drain(gens):
        while gens:
            advance(gens)

    # ---- pipelined schedule: attention groups of both pairs, with QKV-B
    # and proj units as PE filler inside each group's exp-bound steps ----
    fill = []

    def attn_with_fill(st, g):
        for _ in gen_attn(st, g):
            advance(fill)

    # phase 1: QKV + V_aug for pair A (gated by xt DMA arrival)
    for tg in range(NG):
        drain([gen_qkv_tg(stA, tg)])

    fill.append(gen_qkv_tg(stB, 0))
    attn_with_fill(stA, 0)
    fill.append(gen_qkv_tg(stB, 1))
    attn_with_fill(stA, 1)
    fill.append(gen_qkv_tg(stB, 2))
    attn_with_fill(stA, 2)
    drain(fill)
    fill.append(gen_qkv_tg(stB, 3))
    attn_with_fill(stB, 0)
    drain(fill)
    fill.append(gen_proj(0))
    attn_with_fill(stB, 1)
    fill.append(gen_proj(1))
    attn_with_fill(stA, 3)
    attn_with_fill(stB, 2)
    fill.append(gen_proj(2))
    attn_with_fill(stB, 3)
    fill.append(gen_proj(3))
    drain(fill)

    ctx.close()


def _build():
    if "nc" in _nc_cache:
        return _nc_cache["nc"]
    nc = bacc.Bacc("TRN2", target_bir_lowering=False, debug=False)
    with tile.TileContext(nc) as tc:
        _emit(tc)
    nc.compile()
    _nc_cache["nc"] = nc
    return nc


def _make_in_maps(x, wq, wk, wv, w_proj):
    import ml_dtypes

    bf = ml_dtypes.bfloat16
    tri = np.triu(np.ones((P, P), dtype=np.float32)).astype(bf)
    ident = np.tile(np.eye(64, dtype=np.float32), (2, 1)).astype(bf)
    ones = np.ones((P, T // KB, 64), dtype=np.float32).astype(bf)

    def part_major(a, blocks):
        # [blocks*P, cols] -> [P, blocks, cols] so each partition's DMA
        # row is one contiguous span.
        cols = a.shape[1]
        return np.ascontiguousarray(
            a.reshape(blocks, P, cols).transpose(1, 0, 2)
        ).astype(bf)

    xts = []
    for b in range(B):
        xT = np.asarray(x[b], np.float32).T  # [C, T]
        xts.append(
            np.ascontiguousarray(
                xT.reshape(NPO, P, NG, G).transpose(1, 2, 0, 3)
            ).astype(bf)
        )
    in_maps = []
    for c in range(NCORES):
        b, hp = divmod(c, 4)
        h0 = 4 * hp
        in_maps.append(
            {
                "xt": xts[b],
                "wq2": part_major(
                    np.concatenate([wq[h0 + i] for i in range(4)], axis=1), NPO
                ),
                "wk2": part_major(
                    np.concatenate([wk[h0 + i] for i in range(4)], axis=1), NPO
                ),
                "wv2": part_major(
                    np.concatenate([wv[h0 + i] for i in range(4)], axis=1), NPO
                ),
                "wpt": part_major(
                    np.ascontiguousarray(w_proj[:, 256 * hp : 256 * (hp + 1)].T), 2
                ),
                "tri": tri,
                "ident": ident,
                "ones": ones,
            }
        )
    return in_maps


def kernel(x, wq, wk, wv, w_proj, b_proj):
    x = np.asarray(x, dtype=np.float32)
    wq = np.asarray(wq, dtype=np.float32)
    wk = np.asarray(wk, dtype=np.float32)
    wv = np.asarray(wv, dtype=np.float32)
    w_proj = np.asarray(w_proj, dtype=np.float32)
    b_proj = np.asarray(b_proj, dtype=np.float32)

    nc = _build()
    in_maps = _make_in_maps(x, wq, wk, wv, w_proj)
    res = run_bass_kernel_spmd(nc, in_maps, core_ids=list(range(NCORES)))
    acc = np.zeros((B, C, T), dtype=np.float64)
    for c, r in enumerate(res.results):
        acc[c // 4] += r["out"]
    return (acc.transpose(0, 2, 1) + b_proj).astype(np.float32)


# revision 11
# speedup vs baseline: 1.4465x; 1.0014x over previous
"""Multi-head causal attention (B=2, T=2048, C=1024, H=16, HS=64) on 8 TRN2
NeuronCores.

Sharding: hybrid batch x head-group tensor parallel. Core c handles batch
c//4 and heads 4*(c%4) .. 4*(c%4)+3, processed as two head-PAIRS (A, B)
that are software-pipelined against each other. Each core loads only its
batch's activations once (bf16, SBUF-resident), computes a partial output
out_t[C, T] via its 256 rows of w_proj, and the host sums 4 partials per
batch (+ bias) and transposes.

Per-core kernel (bf16 matmuls, fp32 PSUM accumulate):
  - ~12 warm-up matmuls on the first-arriving weight tile keep the PE HAM
    clock at 2.4 GHz through the initial DMA fill; a dummy exp preloads
    the ScalarE activation table.
  - QT/KT/VT [128(2 heads x 64), T] per pair: lhsT=weight chunks (bf16),
    rhs=resident xT chunks.
  - V_aug [keys, 128]: V (cols 0:64 via PE-transpose of VT) | ones.
  - Flash-style causal attention in transposed layout: S^T[keys, q] via
    lhsT=KT block (K=64; the two heads' matmuls occupy disjoint PE row
    groups and run concurrently), exp on ScalarE, O^T accumulated via
    lhsT=V_aug — rows 64:128 give the softmax sums. OT for block-pair jg
    is emitted during jg+1 so the PE never head-of-line blocks on exp.
  - Pair B's QKV matmuls are interleaved between pair A's attention
    steps; the (transposed) output projection lhsT=w_projT chunks,
    rhs=OhatT is interleaved between pair B's attention steps.
"""

import math
import sys
from contextlib import ExitStack

if "/opt/trn_rl_repo" not in sys.path:
    sys.path.insert(0, "/opt/trn_rl_repo")

import numpy as np

import concourse.mybir as mybir
import concourse.tile as tile
from concourse import bacc
from concourse.bass import ts
from concourse.bass_utils import run_bass_kernel_spmd

B, T, C = 2, 2048, 1024
H, HS = 16, 64
NCORES = 8
P = 128
G = 512  # q-group size
NG = T // G
KB = 128  # key block
NPO = C // P  # contraction chunks
F32 = mybir.dt.float32
BF16 = mybir.dt.bfloat16
EXP = mybir.ActivationFunctionType.Exp
MULT = mybir.AluOpType.mult

_nc_cache = {}


def _emit(tc):
    nc = tc.nc
    # All inputs are host-pre-arranged so every DMA descriptor moves a
    # 4-32 KB contiguous row (descriptor-count, not bandwidth, limits the
    # small strided layout).
    xt = nc.dram_tensor("xt", [P, NG, NPO, G], BF16, kind="ExternalInput").ap()
    wq2 = nc.dram_tensor("wq2", [P, NPO, 2 * P], BF16, kind="ExternalInput").ap()
    wk2 = nc.dram_tensor("wk2", [P, NPO, 2 * P], BF16, kind="ExternalInput").ap()
    wv2 = nc.dram_tensor("wv2", [P, NPO, 2 * P], BF16, kind="ExternalInput").ap()
    wptd = nc.dram_tensor("wpt", [P, 2, C], BF16, kind="ExternalInput").ap()
    trid = nc.dram_tensor("tri", [P, P], BF16, kind="ExternalInput").ap()
    identd = nc.dram_tensor("ident", [P, 64], BF16, kind="ExternalInput").ap()
    onesd = nc.dram_tensor("ones", [P, T // KB, 64], BF16, kind="ExternalInput").ap()
    out = nc.dram_tensor("out", [C, T], F32, kind="ExternalOutput").ap()

    ctx = ExitStack()
    persist = ctx.enter_context(tc.tile_pool(name="persist", bufs=1))
    qk_pool = ctx.enter_context(tc.tile_pool(name="qkp", bufs=2))
    vt_pool = ctx.enter_context(tc.tile_pool(name="vtp", bufs=2))
    vaug_pool = ctx.enter_context(tc.tile_pool(name="vaugp", bufs=2))
    pt_pool = ctx.enter_context(tc.tile_pool(name="ptp", bufs=3))
    norm_pool = ctx.enter_context(tc.tile_pool(name="normp", bufs=2))
    ohat_pool = ctx.enter_context(tc.tile_pool(name="ohatp", bufs=2))
    out_pool = ctx.enter_context(tc.tile_pool(name="outp", bufs=4))
    st_psum = ctx.enter_context(tc.tile_pool(name="stps", bufs=2, space="PSUM"))
    ot_psum = ctx.enter_context(tc.tile_pool(name="otps", bufs=2, space="PSUM"))
    mm_psum = ctx.enter_context(tc.tile_pool(name="mmps", bufs=2, space="PSUM"))

    xt_sb = persist.tile([P, NG, NPO, G], BF16, tag="xt")
    wq_sb = persist.tile([P, NPO, 2 * P], BF16, tag="wq")
    wk_sb = persist.tile([P, NPO, 2 * P], BF16, tag="wk")
    wv_sb = persist.tile([P, NPO, 2 * P], BF16, tag="wv")
    wpt_sb = persist.tile([P, 2, C], BF16, tag="wpt")
    tri_sb = persist.tile([P, P], BF16, tag="tri")
    ident = persist.tile([P, 64], BF16, tag="ident")

    # ---- DMA schedule: wq first (enables warm-up), then xt chunks ----
    nc.sync.dma_start(wq_sb[:], wq2[:])

    # PE warm-up during the DMA fill: keeps the HAM clock releasing to
    # 2.4 GHz before the first real matmul. Also preload the exp table.
    warm_ps = mm_psum.tile([P, G], F32, tag="mm", name="warm")
    for _ in range(12):
        nc.tensor.matmul(
            warm_ps[:], wq_sb[:, 0, 0:P], wq_sb[:, 0:2, :], start=True, stop=True
        )
    dummy_act = norm_pool.tile([P, 8], F32, tag="dact", name="dummy_act")
    nc.scalar.activation(dummy_act[:], wq_sb[:, 0, 0:8], EXP, scale=0.01)

    nc.sync.dma_start(xt_sb[:, 0, :, :], xt[:, 0, :, :])
    nc.sync.dma_start(wk_sb[:], wk2[:])
    nc.sync.dma_start(wv_sb[:], wv2[:])
    for tg in range(1, NG):
        nc.sync.dma_start(xt_sb[:, tg, :, :], xt[:, tg, :, :])
    nc.gpsimd.dma_start(ident[:], identd[:])
    nc.gpsimd.dma_start(tri_sb[:], trid[:])

    def new_pair(p):
        st = {
            "p": p,
            "qt": qk_pool.tile([P, T], BF16, tag="qt", name=f"qt{p}"),
            "kt": qk_pool.tile([P, T], BF16, tag="kt", name=f"kt{p}"),
            "vt": vt_pool.tile([P, T], BF16, tag="vt", name=f"vt{p}"),
            "ohat": ohat_pool.tile([P, T], BF16, tag="ohat", name=f"oh{p}"),
            "vaug": [],
        }
        for h in range(2):
            va = vaug_pool.tile(
                [P, T // KB, 128], BF16, tag=f"vaug{h}", name=f"va{p}{h}"
            )
            nc.gpsimd.dma_start(va[:, :, 64:128], onesd[:])
            st["vaug"].append(va)
        return st

    stA = new_pair(0)
    stB = new_pair(1)
    nc.gpsimd.dma_start(wpt_sb[:], wptd[:])

    def emit_qkv(st, which, tg):
        w_sb, dst = {
            "q": (wq_sb, st["qt"]),
            "k": (wk_sb, st["kt"]),
            "v": (wv_sb, st["vt"]),
        }[which]
        sl = slice(P * st["p"], P * st["p"] + P)
        ps = mm_psum.tile([P, G], F32, tag="mm", name=f"qkv{which}{tg}")
        for po in range(NPO):
            nc.tensor.matmul(
                ps[:],
                w_sb[:, po, sl],
                xt_sb[:, tg, po, :],
                start=(po == 0),
                stop=(po == NPO - 1),
            )
        if which == "v":
            nc.scalar.copy(dst[:, ts(tg, G)], ps[:])
        else:
            nc.vector.tensor_copy(dst[:, ts(tg, G)], ps[:])

    def emit_vaug(st, tg):
        tps = [
            mm_psum.tile([P, 4, 64], BF16, tag="mm", name=f"vtr{h}")
            for h in range(2)
        ]
        for kk in range(4):
            kb = 4 * tg + kk
            for h in range(2):
                nc.tensor.transpose(
                    tps[h][:, kk, :],
                    st["vt"][64 * h : 64 * h + 64, ts(kb, KB)],
                    ident[64 * h : 64 * h + 64, :],
                )
        for h in range(2):
            nc.vector.tensor_copy(st["vaug"][h][:, 4 * tg : 4 * tg + 4, 0:64], tps[h][:])

    def emit_ot(st, g, otps, js, pt_h):
        n_j = 4 * g + 4
        for idx, j in enumerate(js):
            r = j - 4 * g
            q0 = P * r if r >= 0 else 0
            for h in range(2):
                nc.tensor.matmul(
                    otps[h][:, q0:G],
                    st["vaug"][h][:, j, :],
                    pt_h[h][:, idx, q0:G],
                    start=(j == 0),
                    stop=(j == n_j - 1),
                )

    def gen_attn(st, g):
        p, qt, kt, ohat = st["p"], st["qt"], st["kt"], st["ohat"]
        n_j = 4 * g + 4
        njg = (n_j + 1) // 2
        otps = [
            ot_psum.tile([P, G], F32, tag="ot", name=f"ot{p}{g}{h}") for h in range(2)
        ]
        prev = None
        for jg in range(njg):
            js = [j for j in (2 * jg, 2 * jg + 1) if j < n_j]
            diag = 2 * jg >= 4 * g
            stps_h = [
                st_psum.tile([P, 2, G], F32, tag="st", name=f"st{p}{g}{h}")
                for h in range(2)
            ]
            pt_h = [
                pt_pool.tile([P, 2, G], BF16, tag=f"pt{h}", name=f"pt{p}{g}{h}")
                for h in range(2)
            ]
            for idx, j in enumerate(js):
                r = j - 4 * g
                q0 = P * r if r >= 0 else 0
                for h in range(2):
                    hb = 64 * h
                    nc.tensor.matmul(
                        stps_h[h][:, idx, q0:G],
                        kt[hb : hb + 64, ts(j, KB)],
                        qt[hb : hb + 64, G * g + q0 : G * (g + 1)],
                        start=True,
                        stop=True,
                    )
            for h in range(2):
                if not diag:
                    nc.scalar.activation(
                        pt_h[h][:, :, :],
                        stps_h[h][:, :, :],
                        EXP,
                        scale=float(HS) ** -0.5,
                    )
                else:
                    for idx, j in enumerate(js):
                        q0 = P * (j - 4 * g)
                        nc.scalar.activation(
                            pt_h[h][:, idx, q0:G],
                            stps_h[h][:, idx, q0:G],
                            EXP,
                            scale=float(HS) ** -0.5,
                        )
                        nc.vector.tensor_tensor(
                            pt_h[h][:, idx, q0 : q0 + P],
                            pt_h[h][:, idx, q0 : q0 + P],
                            tri_sb[:],
                            MULT,
                        )
            if prev is not None:
                emit_ot(st, g, otps, *prev)
            prev = (js, pt_h)
            yield
        emit_ot(st, g, otps, *prev)
        # normalize straight out of the OT PSUM (no staging copies)
        rinv = norm_pool.tile([P, G], F32, tag="rinv", name=f"r{p}{g}")
        for h in range(2):
            hb = 64 * h
            nc.vector.reciprocal_approx_fast(rinv[hb : hb + 64, :], otps[h][64:128, :])
        for h in range(2):
            hb = 64 * h
            nc.vector.tensor_tensor(
                ohat[hb : hb + 64, ts(g, G)], otps[h][0:64, :], rinv[hb : hb + 64, :], MULT
            )
        yield

    def gen_qkv_tg(st, tg):
        for which in ("q", "k", "v"):
            emit_qkv(st, which, tg)
            yield
        emit_vaug(st, tg)
        yield

    def gen_proj(tg):
        for cc in range(NPO):
            ps = mm_psum.tile([P, G], F32, tag="mm", name=f"pj{tg}{cc}")
            for pp in range(2):
                nc.tensor.matmul(
                    ps[:],
                    wpt_sb[:, pp, ts(cc, P)],
                    [stA, stB][pp]["ohat"][:, ts(tg, G)],
                    start=(pp == 0),
                    stop=(pp == 1),
                )
            o_sb = out_pool.tile([P, G], F32, tag="osb", name=f"osb{tg}{cc}")
            if cc % 2 == 0:
                nc.vector.tensor_copy(o_sb[:], ps[:])
                nc.sync.dma_start(out[ts(cc, P), ts(tg, G)], o_sb[:])
            else:
                nc.scalar.copy(o_sb[:], ps[:])
                nc.gpsimd.dma_start(out[ts(cc, P), ts(tg, G)], o_sb[:])
            if cc % 2 == 1:
                yield

    def advance(gens):
        while gens:
            try:
                next(gens[0])
                return
            except StopIteration:
                gens.pop(0)

    def drain(gens):
        while gens:
            advance(gens)

    # ---- pipelined schedule: attention groups start as soon as their
    # inputs exist; later QKV and proj units serve as PE filler inside
    # each group's exp-bound steps. Every group's prerequisite QKV units
    # are force-drained before the group starts. ----
    fill = []

    def attn_with_fill(st, g):
        for _ in gen_attn(st, g):
            advance(fill)

    drain([gen_qkv_tg(stA, 0)])
    fill.append(gen_qkv_tg(stA, 1))
    attn_with_fill(stA, 0)
    fill.append(gen_qkv_tg(stA, 2))
    attn_with_fill(stA, 1)
    fill.append(gen_qkv_tg(stA, 3))
    attn_with_fill(stA, 2)
    drain(fill)
    fill.append(gen_qkv_tg(stB, 0))
    fill.append(gen_qkv_tg(stB, 1))
    attn_with_fill(stA, 3)
    drain(fill)
    fill.append(gen_qkv_tg(stB, 2))
    attn_with_fill(stB, 0)
    drain(fill)
    fill.append(gen_qkv_tg(stB, 3))
    fill.append(gen_proj(0))
    attn_with_fill(stB, 1)
    drain(fill)
    fill.append(gen_proj(1))
    attn_with_fill(stB, 2)
    fill.append(gen_proj(2))
    attn_with_fill(stB, 3)
    fill.append(gen_proj(3))
    drain(fill)

    ctx.close()


def _build():
    if "nc" in _nc_cache:
        return _nc_cache["nc"]
    nc = bacc.Bacc("TRN2", target_bir_lowering=False, debug=False)
    with tile.TileContext(nc) as tc:
        _emit(tc)
    nc.compile()
    _nc_cache["nc"] = nc
    return nc


def _make_in_maps(x, wq, wk, wv, w_proj):
    import ml_dtypes

    bf = ml_dtypes.bfloat16
    tri = np.triu(np.ones((P, P), dtype=np.float32)).astype(bf)
    ident = np.tile(np.eye(64, dtype=np.float32), (2, 1)).astype(bf)
    ones = np.ones((P, T // KB, 64), dtype=np.float32).astype(bf)

    def part_major(a, blocks):
        # [blocks*P, cols] -> [P, blocks, cols] so each partition's DMA
        # row is one contiguous span.
        cols = a.shape[1]
        return np.ascontiguousarray(
            a.reshape(blocks, P, cols).transpose(1, 0, 2)
        ).astype(bf)

    xts = []
    for b in range(B):
        xT = np.asarray(x[b], np.float32).T  # [C, T]
        xts.append(
            np.ascontiguousarray(
                xT.reshape(NPO, P, NG, G).transpose(1, 2, 0, 3)
            ).astype(bf)
        )
    in_maps = []
    for c in range(NCORES):
        b, hp = divmod(c, 4)
        h0 = 4 * hp
        in_maps.append(
            {
                "xt": xts[b],
                "wq2": part_major(
                    np.concatenate([wq[h0 + i] for i in range(4)], axis=1), NPO
                ),
                "wk2": part_major(
                    np.concatenate([wk[h0 + i] for i in range(4)], axis=1), NPO
                ),
                "wv2": part_major(
                    np.concatenate([wv[h0 + i] for i in range(4)], axis=1), NPO
                ),
                "wpt": part_major(
                    np.ascontiguousarray(w_proj[:, 256 * hp : 256 * (hp + 1)].T), 2
                ),
                "tri": tri,
                "ident": ident,
                "ones": ones,
            }
        )
    return in_maps


def kernel(x, wq, wk, wv, w_proj, b_proj):
    x = np.asarray(x, dtype=np.float32)
    wq = np.asarray(wq, dtype=np.float32)
    wk = np.asarray(wk, dtype=np.float32)
    wv = np.asarray(wv, dtype=np.float32)
    w_proj = np.asarray(w_proj, dtype=np.float32)
    b_proj = np.asarray(b_proj, dtype=np.float32)

    nc = _build()
    in_maps = _make_in_maps(x, wq, wk, wv, w_proj)
    res = run_bass_kernel_spmd(nc, in_maps, core_ids=list(range(NCORES)))
    acc = np.zeros((B, C, T), dtype=np.float64)
    for c, r in enumerate(res.results):
        acc[c // 4] += r["out"]
    return (acc.transpose(0, 2, 1) + b_proj).astype(np.float32)


# revision 16
# speedup vs baseline: 1.5085x; 1.0428x over previous
"""Multi-head causal attention (B=2, T=2048, C=1024, H=16, HS=64) on 8 TRN2
NeuronCores.

Sharding: hybrid batch x head-group tensor parallel. Core c handles batch
c//4 and heads 4*(c%4) .. 4*(c%4)+3, processed as two head-PAIRS (A, B)
that are software-pipelined against each other. Each core loads only its
batch's activations once (bf16, SBUF-resident), computes a partial output
out_t[C, T] via its 256 rows of w_proj, and the host sums 4 partials per
batch (+ bias) and transposes.

Per-core kernel (bf16 matmuls, fp32 PSUM accumulate):
  - ~12 warm-up matmuls on the first-arriving weight tile keep the PE HAM
    clock at 2.4 GHz through the initial DMA fill; a dummy exp preloads
    the ScalarE activation table.
  - QT/KT/VT [128(2 heads x 64), T] per pair: lhsT=weight chunks (bf16),
    rhs=resident xT chunks.
  - V_aug [keys, 128]: V (cols 0:64 via PE-transpose of VT) | ones.
  - Flash-style causal attention in transposed layout: S^T[keys, q] via
    lhsT=KT block (K=64; the two heads' matmuls occupy disjoint PE row
    groups and run concurrently), exp on ScalarE, O^T accumulated via
    lhsT=V_aug — rows 64:128 give the softmax sums. OT for block-pair jg
    is emitted during jg+1 so the PE never head-of-line blocks on exp.
  - Pair B's QKV matmuls are interleaved between pair A's attention
    steps; the (transposed) output projection lhsT=w_projT chunks,
    rhs=OhatT is interleaved between pair B's attention steps.
"""

import math
import sys
from contextlib import ExitStack

if "/opt/trn_rl_repo" not in sys.path:
    sys.path.insert(0, "/opt/trn_rl_repo")

import numpy as np

import concourse.mybir as mybir
import concourse.tile as tile
from concourse import bacc
from concourse.bass import ts
from concourse.bass_utils import run_bass_kernel_spmd

B, T, C = 2, 2048, 1024
H, HS = 16, 64
NCORES = 8
P = 128
G = 512  # q-group size
NG = T // G
KB = 128  # key block
NPO = C // P  # contraction chunks
F32 = mybir.dt.float32
BF16 = mybir.dt.bfloat16
EXP = mybir.ActivationFunctionType.Exp
MULT = mybir.AluOpType.mult

_nc_cache = {}


def _emit(tc):
    nc = tc.nc
    # All inputs are host-pre-arranged so every DMA descriptor moves a
    # 4-32 KB contiguous row (descriptor-count, not bandwidth, limits the
    # small strided layout).
    xt = nc.dram_tensor("xt", [P, NG, NPO, G], BF16, kind="ExternalInput").ap()
    wq2 = nc.dram_tensor("wq2", [P, NPO, 2 * P], BF16, kind="ExternalInput").ap()
    wk2 = nc.dram_tensor("wk2", [P, NPO, 2 * P], BF16, kind="ExternalInput").ap()
    wv2 = nc.dram_tensor("wv2", [P, NPO, 2 * P], BF16, kind="ExternalInput").ap()
    wptd = nc.dram_tensor("wpt", [P, 2, C], BF16, kind="ExternalInput").ap()
    trid = nc.dram_tensor("tri", [P, P], BF16, kind="ExternalInput").ap()
    identd = nc.dram_tensor("ident", [P, 64], BF16, kind="ExternalInput").ap()
    onesd = nc.dram_tensor("ones", [P, T // KB, 64], BF16, kind="ExternalInput").ap()
    out = nc.dram_tensor("out", [C, T], F32, kind="ExternalOutput").ap()

    ctx = ExitStack()
    persist = ctx.enter_context(tc.tile_pool(name="persist", bufs=1))
    qk_pool = ctx.enter_context(tc.tile_pool(name="qkp", bufs=2))
    vt_pool = ctx.enter_context(tc.tile_pool(name="vtp", bufs=2))
    vaug_pool = ctx.enter_context(tc.tile_pool(name="vaugp", bufs=2))
    pt_pool = ctx.enter_context(tc.tile_pool(name="ptp", bufs=3))
    norm_pool = ctx.enter_context(tc.tile_pool(name="normp", bufs=2))
    ohat_pool = ctx.enter_context(tc.tile_pool(name="ohatp", bufs=2))
    out_pool = ctx.enter_context(tc.tile_pool(name="outp", bufs=4))
    st_psum = ctx.enter_context(tc.tile_pool(name="stps", bufs=2, space="PSUM"))
    ot_psum = ctx.enter_context(tc.tile_pool(name="otps", bufs=2, space="PSUM"))
    mm_psum = ctx.enter_context(tc.tile_pool(name="mmps", bufs=2, space="PSUM"))

    xt_sb = persist.tile([P, NG, NPO, G], BF16, tag="xt")
    wq_sb = persist.tile([P, NPO, 2 * P], BF16, tag="wq")
    wk_sb = persist.tile([P, NPO, 2 * P], BF16, tag="wk")
    wv_sb = persist.tile([P, NPO, 2 * P], BF16, tag="wv")
    wpt_sb = persist.tile([P, 2, C], BF16, tag="wpt")
    tri_sb = persist.tile([P, P], BF16, tag="tri")
    ident = persist.tile([P, 64], BF16, tag="ident")

    # ---- DMA schedule: a sliver of wq first (enables warm-up), then xt ----
    nc.sync.dma_start(wq_sb[:, 0:2, :], wq2[:, 0:2, :])

    # PE warm-up during the DMA fill: keeps the HAM clock releasing to
    # 2.4 GHz before the first real matmul. Also preload the exp table.
    warm_ps = mm_psum.tile([P, G], F32, tag="mm", name="warm")
    for _ in range(12):
        nc.tensor.matmul(
            warm_ps[:], wq_sb[:, 0, 0:P], wq_sb[:, 0:2, :], start=True, stop=True
        )
    dummy_act = norm_pool.tile([P, 8], F32, tag="dact", name="dummy_act")
    nc.scalar.activation(dummy_act[:], wq_sb[:, 0, 0:8], EXP, scale=0.01)

    nc.sync.dma_start(xt_sb[:, 0, :, :], xt[:, 0, :, :])
    nc.sync.dma_start(wq_sb[:, 2:NPO, :], wq2[:, 2:NPO, :])
    nc.sync.dma_start(wk_sb[:], wk2[:])
    nc.sync.dma_start(wv_sb[:], wv2[:])
    for tg in range(1, NG):
        nc.sync.dma_start(xt_sb[:, tg, :, :], xt[:, tg, :, :])
    nc.gpsimd.dma_start(ident[:], identd[:])
    nc.gpsimd.dma_start(tri_sb[:], trid[:])

    def new_pair(p):
        st = {
            "p": p,
            "qt": qk_pool.tile([P, T], BF16, tag="qt", name=f"qt{p}"),
            "kt": qk_pool.tile([P, T], BF16, tag="kt", name=f"kt{p}"),
            "vt": vt_pool.tile([P, T], BF16, tag="vt", name=f"vt{p}"),
            "ohat": ohat_pool.tile([P, T], BF16, tag="ohat", name=f"oh{p}"),
            "vaug": [],
        }
        for h in range(2):
            va = vaug_pool.tile(
                [P, T // KB, 128], BF16, tag=f"vaug{h}", name=f"va{p}{h}"
            )
            nc.gpsimd.dma_start(va[:, :, 64:128], onesd[:])
            st["vaug"].append(va)
        return st

    stA = new_pair(0)
    stB = new_pair(1)
    nc.gpsimd.dma_start(wpt_sb[:], wptd[:])

    def emit_qkv(st, which, tg):
        w_sb, dst = {
            "q": (wq_sb, st["qt"]),
            "k": (wk_sb, st["kt"]),
            "v": (wv_sb, st["vt"]),
        }[which]
        sl = slice(P * st["p"], P * st["p"] + P)
        ps = mm_psum.tile([P, G], F32, tag="mm", name=f"qkv{which}{tg}")
        for po in range(NPO):
            nc.tensor.matmul(
                ps[:],
                w_sb[:, po, sl],
                xt_sb[:, tg, po, :],
                start=(po == 0),
                stop=(po == NPO - 1),
            )
        if which == "v":
            nc.scalar.copy(dst[:, ts(tg, G)], ps[:])
        else:
            nc.vector.tensor_copy(dst[:, ts(tg, G)], ps[:])

    def emit_vaug(st, tg):
        tps = [
            mm_psum.tile([P, 4, 64], BF16, tag="mm", name=f"vtr{h}")
            for h in range(2)
        ]
        for kk in range(4):
            kb = 4 * tg + kk
            for h in range(2):
                nc.tensor.transpose(
                    tps[h][:, kk, :],
                    st["vt"][64 * h : 64 * h + 64, ts(kb, KB)],
                    ident[64 * h : 64 * h + 64, :],
                )
        for h in range(2):
            nc.vector.tensor_copy(st["vaug"][h][:, 4 * tg : 4 * tg + 4, 0:64], tps[h][:])

    def emit_ot(st, g, otps, js, pt_h):
        n_j = 4 * g + 4
        for idx, j in enumerate(js):
            r = j - 4 * g
            q0 = P * r if r >= 0 else 0
            for h in range(2):
                nc.tensor.matmul(
                    otps[h][:, q0:G],
                    st["vaug"][h][:, j, :],
                    pt_h[h][:, idx, q0:G],
                    start=(j == 0),
                    stop=(j == n_j - 1),
                )

    def gen_attn(st, g):
        p, qt, kt, ohat = st["p"], st["qt"], st["kt"], st["ohat"]
        n_j = 4 * g + 4
        njg = (n_j + 1) // 2
        otps = [
            ot_psum.tile([P, G], F32, tag="ot", name=f"ot{p}{g}{h}") for h in range(2)
        ]
        prev = None
        for jg in range(njg):
            js = [j for j in (2 * jg, 2 * jg + 1) if j < n_j]
            diag = 2 * jg >= 4 * g
            stps_h = [
                st_psum.tile([P, 2, G], F32, tag="st", name=f"st{p}{g}{h}")
                for h in range(2)
            ]
            pt_h = [
                pt_pool.tile([P, 2, G], BF16, tag=f"pt{h}", name=f"pt{p}{g}{h}")
                for h in range(2)
            ]
            for idx, j in enumerate(js):
                r = j - 4 * g
                q0 = P * r if r >= 0 else 0
                for h in range(2):
                    hb = 64 * h
                    nc.tensor.matmul(
                        stps_h[h][:, idx, q0:G],
                        kt[hb : hb + 64, ts(j, KB)],
                        qt[hb : hb + 64, G * g + q0 : G * (g + 1)],
                        start=True,
                        stop=True,
                    )
            for h in range(2):
                if not diag:
                    nc.scalar.activation(
                        pt_h[h][:, :, :],
                        stps_h[h][:, :, :],
                        EXP,
                        scale=float(HS) ** -0.5,
                    )
                else:
                    for idx, j in enumerate(js):
                        q0 = P * (j - 4 * g)
                        nc.scalar.activation(
                            pt_h[h][:, idx, q0:G],
                            stps_h[h][:, idx, q0:G],
                            EXP,
                            scale=float(HS) ** -0.5,
                        )
                        nc.vector.tensor_tensor(
                            pt_h[h][:, idx, q0 : q0 + P],
                            pt_h[h][:, idx, q0 : q0 + P],
                            tri_sb[:],
                            MULT,
                        )
            if prev is not None:
                emit_ot(st, g, otps, *prev)
            prev = (js, pt_h)
            yield
        emit_ot(st, g, otps, *prev)
        # normalization: partition-shifting copies (DVE/ACT tensor_copy can
        # cross partitions; lane-wise ALU ops cannot), then aligned ops.
        l_sb = norm_pool.tile([P, G], F32, tag="lsb", name=f"l{p}{g}")
        rinv = norm_pool.tile([P, G], F32, tag="rinv", name=f"r{p}{g}")
        stag = norm_pool.tile([P, G], F32, tag="stag", name=f"sg{p}{g}")
        for h in range(2):
            hb = 64 * h
            nc.scalar.copy(l_sb[hb : hb + 64, :], otps[h][64:128, :])
            nc.vector.tensor_copy(stag[hb : hb + 64, :], otps[h][0:64, :])
        nc.vector.reciprocal_approx_fast(rinv[:], l_sb[:])
        nc.vector.tensor_tensor(ohat[:, ts(g, G)], stag[:], rinv[:], MULT)
        yield

    def gen_qkv_tg(st, tg):
        for which in ("q", "k", "v"):
            emit_qkv(st, which, tg)
            yield
        emit_vaug(st, tg)
        yield

    def gen_proj(tg):
        for cc in range(NPO):
            ps = mm_psum.tile([P, G], F32, tag="mm", name=f"pj{tg}{cc}")
            for pp in range(2):
                nc.tensor.matmul(
                    ps[:],
                    wpt_sb[:, pp, ts(cc, P)],
                    [stA, stB][pp]["ohat"][:, ts(tg, G)],
                    start=(pp == 0),
                    stop=(pp == 1),
                )
            o_sb = out_pool.tile([P, G], F32, tag="osb", name=f"osb{tg}{cc}")
            nc.vector.tensor_copy(o_sb[:], ps[:])
            if cc % 2 == 0:
                nc.sync.dma_start(out[ts(cc, P), ts(tg, G)], o_sb[:])
            else:
                nc.gpsimd.dma_start(out[ts(cc, P), ts(tg, G)], o_sb[:])
            if cc % 2 == 1:
                yield

    def advance(gens):
        while gens:
            try:
                next(gens[0])
                return
            except StopIteration:
                gens.pop(0)

    def drain(gens):
        while gens:
            advance(gens)

    # ---- pipelined schedule: attention groups start as soon as their
    # inputs exist; later QKV and proj units serve as PE filler inside
    # each group's exp-bound steps. Every group's prerequisite QKV units
    # are force-drained before the group starts. ----
    fill = []

    def attn_with_fill(st, g):
        for _ in gen_attn(st, g):
            advance(fill)

    qA = [gen_qkv_tg(stA, tg) for tg in range(NG)]
    qB = [gen_qkv_tg(stB, tg) for tg in range(NG)]
    pj = [gen_proj(tg) for tg in range(NG)]

    drain([qA[0]])
    fill.append(qA[1])
    attn_with_fill(stA, 0)
    drain([qA[1]])
    fill.append(qA[2])
    attn_with_fill(stA, 1)
    drain([qA[2]])
    fill.append(qA[3])
    attn_with_fill(stA, 2)
    drain([qA[3]])
    fill.append(qB[0])
    fill.append(qB[1])
    attn_with_fill(stA, 3)
    drain([qB[0]])
    fill.append(qB[2])
    attn_with_fill(stB, 0)
    drain([qB[1]])
    fill.append(qB[3])
    fill.append(pj[0])
    attn_with_fill(stB, 1)
    drain([qB[2]])
    fill.append(pj[1])
    attn_with_fill(stB, 2)
    drain([qB[3]])
    fill.append(pj[2])
    attn_with_fill(stB, 3)
    fill.append(pj[3])
    drain(fill)

    ctx.close()


def _build():
    if "nc" in _nc_cache:
        return _nc_cache["nc"]
    nc = bacc.Bacc("TRN2", target_bir_lowering=False, debug=False)
    with tile.TileContext(nc) as tc:
        _emit(tc)
    nc.compile()
    _nc_cache["nc"] = nc
    return nc


def _make_in_maps(x, wq, wk, wv, w_proj):
    import ml_dtypes

    bf = ml_dtypes.bfloat16
    tri = np.triu(np.ones((P, P), dtype=np.float32)).astype(bf)
    ident = np.tile(np.eye(64, dtype=np.float32), (2, 1)).astype(bf)
    ones = np.ones((P, T // KB, 64), dtype=np.float32).astype(bf)

    def part_major(a, blocks):
        # [blocks*P, cols] -> [P, blocks, cols] so each partition's DMA
        # row is one contiguous span.
        cols = a.shape[1]
        return np.ascontiguousarray(
            a.reshape(blocks, P, cols).transpose(1, 0, 2)
        ).astype(bf)

    xts = []
    for b in range(B):
        xT = np.asarray(x[b], np.float32).T  # [C, T]
        xts.append(
            np.ascontiguousarray(
                xT.reshape(NPO, P, NG, G).transpose(1, 2, 0, 3)
            ).astype(bf)
        )
    in_maps = []
    for c in range(NCORES):
        b, hp = divmod(c, 4)
        h0 = 4 * hp
        in_maps.append(
            {
                "xt": xts[b],
                "wq2": part_major(
                    np.concatenate([wq[h0 + i] for i in range(4)], axis=1), NPO
                ),
                "wk2": part_major(
                    np.concatenate([wk[h0 + i] for i in range(4)], axis=1), NPO
                ),
                "wv2": part_major(
                    np.concatenate([wv[h0 + i] for i in range(4)], axis=1), NPO
                ),
                "wpt": part_major(
                    np.ascontiguousarray(w_proj[:, 256 * hp : 256 * (hp + 1)].T), 2
                ),
                "tri": tri,
                "ident": ident,
                "ones": ones,
            }
        )
    return in_maps


def kernel(x, wq, wk, wv, w_proj, b_proj):
    x = np.asarray(x, dtype=np.float32)
    wq = np.asarray(wq, dtype=np.float32)
    wk = np.asarray(wk, dtype=np.float32)
    wv = np.asarray(wv, dtype=np.float32)
    w_proj = np.asarray(w_proj, dtype=np.float32)
    b_proj = np.asarray(b_proj, dtype=np.float32)

    nc = _build()
    in_maps = _make_in_maps(x, wq, wk, wv, w_proj)
    res = run_bass_kernel_spmd(nc, in_maps, core_ids=list(range(NCORES)))
    acc = np.zeros((B, C, T), dtype=np.float64)
    for c, r in enumerate(res.results):
        acc[c // 4] += r["out"]
    return (acc.transpose(0, 2, 1) + b_proj).astype(np.float32)


# revision 18
# speedup vs baseline: 1.5355x; 1.0179x over previous
"""Multi-head causal attention (B=2, T=2048, C=1024, H=16, HS=64) on 8 TRN2
NeuronCores.

Sharding: hybrid batch x head-group tensor parallel. Core c handles batch
c//4 and heads 4*(c%4) .. 4*(c%4)+3, processed as two head-PAIRS (A, B)
that are software-pipelined against each other. Each core loads only its
batch's activations once (bf16, SBUF-resident), computes a partial output
out_t[C, T] via its 256 rows of w_proj, and the host sums 4 partials per
batch (+ bias) and transposes.

Per-core kernel (bf16 matmuls, fp32 PSUM accumulate):
  - ~12 warm-up matmuls on the first-arriving weight tile keep the PE HAM
    clock at 2.4 GHz through the initial DMA fill; a dummy exp preloads
    the ScalarE activation table.
  - QT/KT/VT [128(2 heads x 64), T] per pair: lhsT=weight chunks (bf16),
    rhs=resident xT chunks.
  - V_aug [keys, 128]: V (cols 0:64 via PE-transpose of VT) | ones.
  - Flash-style causal attention in transposed layout: S^T[keys, q] via
    lhsT=KT block (K=64; the two heads' matmuls occupy disjoint PE row
    groups and run concurrently), exp on ScalarE, O^T accumulated via
    lhsT=V_aug — rows 64:128 give the softmax sums. OT for block-pair jg
    is emitted during jg+1 so the PE never head-of-line blocks on exp.
  - Pair B's QKV matmuls are interleaved between pair A's attention
    steps; the (transposed) output projection lhsT=w_projT chunks,
    rhs=OhatT is interleaved between pair B's attention steps.
"""

import math
import sys
from contextlib import ExitStack

if "/opt/trn_rl_repo" not in sys.path:
    sys.path.insert(0, "/opt/trn_rl_repo")

import numpy as np

import concourse.mybir as mybir
import concourse.tile as tile
from concourse import bacc
from concourse.bass import ts
from concourse.bass_utils import run_bass_kernel_spmd

B, T, C = 2, 2048, 1024
H, HS = 16, 64
NCORES = 8
P = 128
G = 512  # q-group size
NG = T // G
KB = 128  # key block
NPO = C // P  # contraction chunks
F32 = mybir.dt.float32
BF16 = mybir.dt.bfloat16
EXP = mybir.ActivationFunctionType.Exp
MULT = mybir.AluOpType.mult

_nc_cache = {}


def _emit(tc):
    nc = tc.nc
    # All inputs are host-pre-arranged so every DMA descriptor moves a
    # 4-32 KB contiguous row (descriptor-count, not bandwidth, limits the
    # small strided layout).
    xt = nc.dram_tensor("xt", [P, NG, NPO, G], BF16, kind="ExternalInput").ap()
    wq2 = nc.dram_tensor("wq2", [P, NPO, 2 * P], BF16, kind="ExternalInput").ap()
    wk2 = nc.dram_tensor("wk2", [P, NPO, 2 * P], BF16, kind="ExternalInput").ap()
    wv2 = nc.dram_tensor("wv2", [P, NPO, 2 * P], BF16, kind="ExternalInput").ap()
    wptd = nc.dram_tensor("wpt", [P, 2, C], BF16, kind="ExternalInput").ap()
    trid = nc.dram_tensor("tri", [P, P], BF16, kind="ExternalInput").ap()
    identd = nc.dram_tensor("ident", [P, 64], BF16, kind="ExternalInput").ap()
    onesd = nc.dram_tensor("ones", [P, T // KB, 64], BF16, kind="ExternalInput").ap()
    out = nc.dram_tensor("out", [C, T], F32, kind="ExternalOutput").ap()

    ctx = ExitStack()
    persist = ctx.enter_context(tc.tile_pool(name="persist", bufs=1))
    qk_pool = ctx.enter_context(tc.tile_pool(name="qkp", bufs=2))
    vt_pool = ctx.enter_context(tc.tile_pool(name="vtp", bufs=2))
    vaug_pool = ctx.enter_context(tc.tile_pool(name="vaugp", bufs=2))
    pt_pool = ctx.enter_context(tc.tile_pool(name="ptp", bufs=3))
    norm_pool = ctx.enter_context(tc.tile_pool(name="normp", bufs=2))
    ohat_pool = ctx.enter_context(tc.tile_pool(name="ohatp", bufs=2))
    out_pool = ctx.enter_context(tc.tile_pool(name="outp", bufs=4))
    st_psum = ctx.enter_context(tc.tile_pool(name="stps", bufs=2, space="PSUM"))
    ot_psum = ctx.enter_context(tc.tile_pool(name="otps", bufs=2, space="PSUM"))
    mm_psum = ctx.enter_context(tc.tile_pool(name="mmps", bufs=2, space="PSUM"))

    xt_sb = persist.tile([P, NG, NPO, G], BF16, tag="xt")
    wq_sb = persist.tile([P, NPO, 2 * P], BF16, tag="wq")
    wk_sb = persist.tile([P, NPO, 2 * P], BF16, tag="wk")
    wv_sb = persist.tile([P, NPO, 2 * P], BF16, tag="wv")
    wpt_sb = persist.tile([P, 2, C], BF16, tag="wpt")
    tri_sb = persist.tile([P, P], BF16, tag="tri")
    ident = persist.tile([P, 64], BF16, tag="ident")

    # ---- DMA schedule: a sliver of wq first (enables warm-up), then xt ----
    nc.sync.dma_start(wq_sb[:, 0:2, :], wq2[:, 0:2, :])

    # PE warm-up during the DMA fill: keeps the HAM clock releasing to
    # 2.4 GHz before the first real matmul. Also preload the exp table.
    warm_ps = mm_psum.tile([P, G], F32, tag="mm", name="warm")
    for _ in range(12):
        nc.tensor.matmul(
            warm_ps[:], wq_sb[:, 0, 0:P], wq_sb[:, 0:2, :], start=True, stop=True
        )
    dummy_act = norm_pool.tile([P, 8], F32, tag="dact", name="dummy_act")
    nc.scalar.activation(dummy_act[:], wq_sb[:, 0, 0:8], EXP, scale=0.01)

    nc.sync.dma_start(xt_sb[:, 0, :, :], xt[:, 0, :, :])
    nc.sync.dma_start(wq_sb[:, 2:NPO, :], wq2[:, 2:NPO, :])
    nc.sync.dma_start(wk_sb[:], wk2[:])
    nc.sync.dma_start(wv_sb[:], wv2[:])
    for tg in range(1, NG):
        nc.sync.dma_start(xt_sb[:, tg, :, :], xt[:, tg, :, :])
    nc.gpsimd.dma_start(ident[:], identd[:])
    nc.gpsimd.dma_start(tri_sb[:], trid[:])

    def new_pair(p):
        st = {
            "p": p,
            "qt": qk_pool.tile([P, T], BF16, tag="qt", name=f"qt{p}"),
            "kt": qk_pool.tile([P, T], BF16, tag="kt", name=f"kt{p}"),
            "vt": vt_pool.tile([P, T], BF16, tag="vt", name=f"vt{p}"),
            "ohat": ohat_pool.tile([P, T], BF16, tag="ohat", name=f"oh{p}"),
            "vaug": [],
        }
        for h in range(2):
            va = vaug_pool.tile(
                [P, T // KB, 128], BF16, tag=f"vaug{h}", name=f"va{p}{h}"
            )
            nc.gpsimd.dma_start(va[:, :, 64:128], onesd[:])
            st["vaug"].append(va)
        return st

    stA = new_pair(0)
    stB = new_pair(1)
    nc.gpsimd.dma_start(wpt_sb[:], wptd[:])

    def emit_qkv(st, which, tg):
        w_sb, dst = {
            "q": (wq_sb, st["qt"]),
            "k": (wk_sb, st["kt"]),
            "v": (wv_sb, st["vt"]),
        }[which]
        sl = slice(P * st["p"], P * st["p"] + P)
        ps = mm_psum.tile([P, G], F32, tag="mm", name=f"qkv{which}{tg}")
        for po in range(NPO):
            nc.tensor.matmul(
                ps[:],
                w_sb[:, po, sl],
                xt_sb[:, tg, po, :],
                start=(po == 0),
                stop=(po == NPO - 1),
            )
        if which == "v":
            nc.scalar.copy(dst[:, ts(tg, G)], ps[:])
        else:
            nc.vector.tensor_copy(dst[:, ts(tg, G)], ps[:])

    def emit_vaug(st, tg):
        tps = [
            mm_psum.tile([P, 4, 64], BF16, tag="mm", name=f"vtr{h}")
            for h in range(2)
        ]
        for kk in range(4):
            kb = 4 * tg + kk
            for h in range(2):
                nc.tensor.transpose(
                    tps[h][:, kk, :],
                    st["vt"][64 * h : 64 * h + 64, ts(kb, KB)],
                    ident[64 * h : 64 * h + 64, :],
                )
        for h in range(2):
            nc.vector.tensor_copy(st["vaug"][h][:, 4 * tg : 4 * tg + 4, 0:64], tps[h][:])

    def emit_ot(st, g, otps, js, pt_h):
        n_j = 4 * g + 4
        for idx, j in enumerate(js):
            r = j - 4 * g
            q0 = P * r if r >= 0 else 0
            for h in range(2):
                nc.tensor.matmul(
                    otps[h][:, q0:G],
                    st["vaug"][h][:, j, :],
                    pt_h[h][:, idx, q0:G],
                    start=(j == 0),
                    stop=(j == n_j - 1),
                )

    def gen_attn(st, g):
        p, qt, kt, ohat = st["p"], st["qt"], st["kt"], st["ohat"]
        n_j = 4 * g + 4
        njg = (n_j + 1) // 2
        otps = [
            ot_psum.tile([P, G], F32, tag="ot", name=f"ot{p}{g}{h}") for h in range(2)
        ]
        prev = None
        for jg in range(njg):
            js = [j for j in (2 * jg, 2 * jg + 1) if j < n_j]
            diag = 2 * jg >= 4 * g
            stps_h = [
                st_psum.tile([P, 2, G], F32, tag="st", name=f"st{p}{g}{h}")
                for h in range(2)
            ]
            pt_h = [
                pt_pool.tile([P, 2, G], BF16, tag=f"pt{h}", name=f"pt{p}{g}{h}")
                for h in range(2)
            ]
            for idx, j in enumerate(js):
                r = j - 4 * g
                q0 = P * r if r >= 0 else 0
                for h in range(2):
                    hb = 64 * h
                    nc.tensor.matmul(
                        stps_h[h][:, idx, q0:G],
                        kt[hb : hb + 64, ts(j, KB)],
                        qt[hb : hb + 64, G * g + q0 : G * (g + 1)],
                        start=True,
                        stop=True,
                    )
            for h in range(2):
                if not diag:
                    nc.scalar.activation(
                        pt_h[h][:, :, :],
                        stps_h[h][:, :, :],
                        EXP,
                        scale=float(HS) ** -0.5,
                    )
                else:
                    for idx, j in enumerate(js):
                        q0 = P * (j - 4 * g)
                        nc.scalar.activation(
                            pt_h[h][:, idx, q0:G],
                            stps_h[h][:, idx, q0:G],
                            EXP,
                            scale=float(HS) ** -0.5,
                        )
                        nc.vector.tensor_tensor(
                            pt_h[h][:, idx, q0 : q0 + P],
                            pt_h[h][:, idx, q0 : q0 + P],
                            tri_sb[:],
                            MULT,
                        )
            if prev is not None:
                emit_ot(st, g, otps, *prev)
            prev = (js, pt_h)
            yield
        emit_ot(st, g, otps, *prev)
        # normalization: partition-shifting copies (DVE/ACT tensor_copy can
        # cross partitions; lane-wise ALU ops cannot), then aligned ops.
        l_sb = norm_pool.tile([P, G], F32, tag="lsb", name=f"l{p}{g}")
        rinv = norm_pool.tile([P, G], F32, tag="rinv", name=f"r{p}{g}")
        stag = norm_pool.tile([P, G], F32, tag="stag", name=f"sg{p}{g}")
        for h in range(2):
            hb = 64 * h
            nc.scalar.copy(l_sb[hb : hb + 64, :], otps[h][64:128, :])
            nc.vector.tensor_copy(stag[hb : hb + 64, :], otps[h][0:64, :])
        nc.vector.reciprocal_approx_fast(rinv[:], l_sb[:])
        nc.vector.tensor_tensor(ohat[:, ts(g, G)], stag[:], rinv[:], MULT)
        yield

    def gen_qkv_tg(st, tg):
        for which in ("q", "k", "v"):
            emit_qkv(st, which, tg)
            yield
        emit_vaug(st, tg)
        yield

    def gen_proj(tg):
        for cc in range(NPO):
            ps = mm_psum.tile([P, G], F32, tag="mm", name=f"pj{tg}{cc}")
            for pp in range(2):
                nc.tensor.matmul(
                    ps[:],
                    wpt_sb[:, pp, ts(cc, P)],
                    [stA, stB][pp]["ohat"][:, ts(tg, G)],
                    start=(pp == 0),
                    stop=(pp == 1),
                )
            o_sb = out_pool.tile([P, G], F32, tag="osb", name=f"osb{tg}{cc}")
            # mid-kernel: keep ScalarE free for exp; at the tail (tg==3)
            # split copies across engines to drain faster.
            if tg == 3 and cc % 2 == 1:
                nc.scalar.copy(o_sb[:], ps[:])
            else:
                nc.vector.tensor_copy(o_sb[:], ps[:])
            if cc % 2 == 0:
                nc.sync.dma_start(out[ts(cc, P), ts(tg, G)], o_sb[:])
            else:
                nc.gpsimd.dma_start(out[ts(cc, P), ts(tg, G)], o_sb[:])
            if cc % 2 == 1:
                yield

    def advance(gens):
        while gens:
            try:
                next(gens[0])
                return
            except StopIteration:
                gens.pop(0)

    def drain(gens):
        while gens:
            advance(gens)

    # ---- pipelined schedule: attention groups start as soon as their
    # inputs exist; later QKV and proj units serve as PE filler inside
    # each group's exp-bound steps. Every group's prerequisite QKV units
    # are force-drained before the group starts. ----
    fill = []

    def attn_with_fill(st, g):
        for _ in gen_attn(st, g):
            advance(fill)

    qA = [gen_qkv_tg(stA, tg) for tg in range(NG)]
    qB = [gen_qkv_tg(stB, tg) for tg in range(NG)]
    pj = [gen_proj(tg) for tg in range(NG)]

    def emit_warm(n):
        # Filler matmuls on already-loaded weights: executed in queue order,
        # they occupy the PE during xt-DMA stalls and keep the HAM clock hot.
        wps = mm_psum.tile([P, G], F32, tag="mm", name="warmf")
        for _ in range(n):
            nc.tensor.matmul(
                wps[:], wq_sb[:, 0, 0:P], wq_sb[:, 0:2, :], start=True, stop=True
            )

    drain([qA[0]])
    emit_warm(8)
    fill.append(qA[1])
    attn_with_fill(stA, 0)
    drain([qA[1]])
    emit_warm(6)
    fill.append(qA[2])
    attn_with_fill(stA, 1)
    drain([qA[2]])
    fill.append(qA[3])
    attn_with_fill(stA, 2)
    drain([qA[3]])
    fill.append(qB[0])
    fill.append(qB[1])
    attn_with_fill(stA, 3)
    drain([qB[0]])
    fill.append(qB[2])
    attn_with_fill(stB, 0)
    drain([qB[1]])
    fill.append(qB[3])
    fill.append(pj[0])
    attn_with_fill(stB, 1)
    drain([qB[2]])
    fill.append(pj[1])
    attn_with_fill(stB, 2)
    drain([qB[3]])
    fill.append(pj[2])
    attn_with_fill(stB, 3)
    fill.append(pj[3])
    drain(fill)

    ctx.close()


def _build():
    if "nc" in _nc_cache:
        return _nc_cache["nc"]
    nc = bacc.Bacc("TRN2", target_bir_lowering=False, debug=False)
    with tile.TileContext(nc) as tc:
        _emit(tc)
    nc.compile()
    _nc_cache["nc"] = nc
    return nc


def _make_in_maps(x, wq, wk, wv, w_proj):
    import ml_dtypes

    bf = ml_dtypes.bfloat16
    tri = np.triu(np.ones((P, P), dtype=np.float32)).astype(bf)
    ident = np.tile(np.eye(64, dtype=np.float32), (2, 1)).astype(bf)
    ones = np.ones((P, T // KB, 64), dtype=np.float32).astype(bf)

    def part_major(a, blocks):
        # [blocks*P, cols] -> [P, blocks, cols] so each partition's DMA
        # row is one contiguous span.
        cols = a.shape[1]
        return np.ascontiguousarray(
            a.reshape(blocks, P, cols).transpose(1, 0, 2)
        ).astype(bf)

    xts = []
    for b in range(B):
        xT = np.asarray(x[b], np.float32).T  # [C, T]
        xts.append(
            np.ascontiguousarray(
                xT.reshape(NPO, P, NG, G).transpose(1, 2, 0, 3)
            ).astype(bf)
        )
    in_maps = []
    for c in range(NCORES):
        b, hp = divmod(c, 4)
        h0 = 4 * hp
        in_maps.append(
            {
                "xt": xts[b],
                "wq2": part_major(
                    np.concatenate([wq[h0 + i] for i in range(4)], axis=1), NPO
                ),
                "wk2": part_major(
                    np.concatenate([wk[h0 + i] for i in range(4)], axis=1), NPO
                ),
                "wv2": part_major(
                    np.concatenate([wv[h0 + i] for i in range(4)], axis=1), NPO
                ),
                "wpt": part_major(
                    np.ascontiguousarray(w_proj[:, 256 * hp : 256 * (hp + 1)].T), 2
                ),
                "tri": tri,
                "ident": ident,
                "ones": ones,
            }
        )
    return in_maps


def kernel(x, wq, wk, wv, w_proj, b_proj):
    x = np.asarray(x, dtype=np.float32)
    wq = np.asarray(wq, dtype=np.float32)
    wk = np.asarray(wk, dtype=np.float32)
    wv = np.asarray(wv, dtype=np.float32)
    w_proj = np.asarray(w_proj, dtype=np.float32)
    b_proj = np.asarray(b_proj, dtype=np.float32)

    nc = _build()
    in_maps = _make_in_maps(x, wq, wk, wv, w_proj)
    res = run_bass_kernel_spmd(nc, in_maps, core_ids=list(range(NCORES)))
    acc = np.zeros((B, C, T), dtype=np.float64)
    for c, r in enumerate(res.results):
        acc[c // 4] += r["out"]
    return (acc.transpose(0, 2, 1) + b_proj).astype(np.float32)
